# revision 19
# baseline (speedup 1.0000x reference)
"""ExpanderConv2d as a Bass/Tile kernel for Trainium2, data-parallel over batch
across 8 NeuronCores.

Reference op: y = conv2d(x, weight * mask), N=32, C=256->256, 56x56, k=3,
stride 1, pad 1.

v5: 1D Winograd F(4,3) along W.  Each quad of 4 horizontal outputs costs 6
multiplies instead of 12, so the PE streams 2/3 the columns of the direct
9-tap formulation: 226k columns/core (~94us at 1 col/cycle) vs 451k (~188us).

Per (icc, img): x [128,56,56] fp16 -> ACT phase-split into 4 column-phase
planes [58 rows, 15 blocks] plus 3 ACT-prescaled planes (4*E0, 4*E1, 4*E2;
exact in fp16) -> V[m] for the 6 Winograd components, mostly as plain
tensor_tensor at the DVE 2x rate (the prescaled planes remove the slow 1x
scalar_tensor_tensor ops) -> GEMMs M[m] = sum_{ky,icc} W'[m,ky,icc]^T V[m]
(PSUM fp32, moving dim = 28 rows x 14 tiles = 392) -> ACT evicts M to SBUF
fp16 -> output transform y[v] = At M on [56,14] slabs -> fp16 phase-planar
DMA; the host interleaves the 4 phases (pure transpose) and upcasts.

Engine discipline learned from traces: GpSimd ops are ~3x slower than DVE
and must never produce values DVE waits on (only leaf work: V3/V4, IJ/yv0/
yv3); the ACT stream is hand-interleaved so M evictions track the PE's
PSUM-bank rotation with next-image phase splits injected in the gaps.

Sharding: batch 32 -> 4 images per core; the transformed masked weight
(2.4 MB fp16, 72 [128x128] tiles) is replicated to every core.
"""

import numpy as np

N_CORES = 8
IMG_PER_CORE = 4
C = 256
H = 56
TX = 14          # winograd tiles per row (4 outputs each)
PHB = 15         # phase-plane blocks (E0/E1 need 15, E2/E3 use 14)
VR = 58          # V rows = padded rows
NW = 72          # weight tiles: occ(2) x m(6) x ky(3) x icc(2)


def _split_waits(nc, max_waits=1):
    """walrus in this container rejects instructions carrying more than one
    semaphore wait ("Too many sync wait commands").  Hoist the extra waits onto
    injected single-wait NoOps on the same engine just before the instruction —
    sem waits block the engine, so a chain of single waits is equivalent."""
    import concourse.mybir as mybir

    for f in nc.m.functions:
        for blk in f.blocks:
            out = []
            changed = False
            for inst in blk.instructions:
                si = inst.sync_info
                if si and si.on_wait and len(si.on_wait) > max_waits:
                    waits = list(si.on_wait)
                    extra, keep = waits[:-max_waits], waits[-max_waits:]
                    for j, w in enumerate(extra):
                        out.append(
                            mybir.InstNoOp(
                                name=f"{inst.name}-w{j}",
                                engine=inst.engine,
                                ins=[],
                                outs=[],
                                sync_info=mybir.SyncInfo(on_wait=[w], on_update=[]),
                                bass_nofuse=True,
                            )
                        )
                    si.on_wait = keep
                    changed = True
                out.append(inst)
            if changed:
                blk.instructions = out


def _build_nc():
    import concourse.bass as bass
    import concourse.mybir as mybir
    from concourse.tile import TileContext

    f32 = mybir.dt.float32
    f16 = mybir.dt.float16
    ADD = mybir.AluOpType.add
    SUB = mybir.AluOpType.subtract
    MUL = mybir.AluOpType.mult

    nc = bass.Bass("TRN2", target_bir_lowering=False, debug=False)
    x_d = nc.dram_tensor("x", [IMG_PER_CORE, C, H, H], f16, kind="ExternalInput").ap()
    w_d = nc.dram_tensor("w", [128, NW * 128], f16, kind="ExternalInput").ap()
    # y is stored phase-planar: y[img, c, v, h, tx] = out[img, c, h, 4*tx+v]
    y_d = nc.dram_tensor("y", [IMG_PER_CORE, C, 4, H, TX], f16, kind="ExternalOutput").ap()

    with TileContext(nc) as tc:
        with (
            tc.tile_pool(name="wpool", bufs=1) as wp,
            tc.tile_pool(name="xpool", bufs=1) as xp,
            tc.tile_pool(name="psum", bufs=8, space="PSUM") as pp,
            tc.tile_pool(name="msb", bufs=1) as mp,
            tc.tile_pool(name="scp", bufs=1) as scp,
        ):
            w_sb = wp.tile([128, NW * 128], f16, name="w_sb", tag="w_sb")
            # Each GEMM group consumes 6 consecutive tiles ((occ,m) x ky x icc);
            # DMA in matching chunks so groups start as their tiles land.
            wq = 0
            for wn in (6, 6, 6, 6, 6, 6, 18, 18):
                nc.scalar.dma_start(
                    out=w_sb[:, wq * 128 : (wq + wn) * 128],
                    in_=w_d[:, wq * 128 : (wq + wn) * 128],
                )
                wq += wn

            # Warm the PE clock gate (HAM) with throwaway matmuls on scratch
            # data while the first input/weight DMAs are in flight.
            warm = wp.tile([128, 392], f16, name="warm", tag="warm")
            nc.vector.memset(warm[:], 0.0)
            for i in range(10):
                warm_ps = pp.tile([128, 28, TX], f32, name="mt", tag="mt")
                nc.tensor.matmul(
                    warm_ps[:], warm[:, :128], warm[:, :392], start=True, stop=True
                )

            # Per (icc, ping/pong): raw input, phase planes (4 raw + 3
            # prescaled), V components.
            xrs = [
                [xp.tile([128, H, H], f16, name=f"xr{i}{b}", tag=f"xr{i}{b}") for b in range(2)]
                for i in range(2)
            ]
            phs = [
                [xp.tile([128, 6, VR, PHB], f16, name=f"ph{i}{b}", tag=f"ph{i}{b}") for b in range(2)]
                for i in range(2)
            ]
            vts = [
                [xp.tile([128, 6, VR, TX], f16, name=f"vt{i}{b}", tag=f"vt{i}{b}") for b in range(2)]
                for i in range(2)
            ]
            # Zero the padding cells of the phase planes once; DMA/split/scale
            # only ever write interior rows/blocks, so they stay zero.
            for i in range(2):
                for b in range(2):
                    ph = phs[i][b]
                    nc.gpsimd.memset(ph[:, :, 0, :], 0.0)        # top pad row
                    nc.gpsimd.memset(ph[:, :, VR - 1, :], 0.0)   # bottom pad row
                    nc.gpsimd.memset(ph[:, 0, :, 0], 0.0)        # left pad col (E0 blk 0)
                    nc.gpsimd.memset(ph[:, 1, :, PHB - 1], 0.0)  # right pad col (E1 blk 14)

            # V scratch, ping-ponged per image: a single shared buffer makes
            # DVE's img N+1 subexpression writes wait on GpSimd's img N reads
            # (observed 10us cross-engine WAR stalls).
            # sc_d slabs: 0:B 1:D 2:Es 3:F 4:w5 5:s 6:t 7:u
            sc_ds = [scp.tile([128, 8, VR, TX], f16, name=f"sc_d{b}", tag=f"sc_d{b}") for b in range(2)]
            sc_g = scp.tile([128, 1, VR, TX], f16, name="sc_g", tag="sc_g")
            msbs = [mp.tile([128, 6, H, TX], f16, name=f"m{b}", tag=f"m{b}") for b in range(2)]
            # out-transform scratch: G2/H2 stay DVE-private (single buffer);
            # slabs read by GpSimd or DMA'd (I,J,y3t,IJ,yv0-3) ping-pong.
            sc2gh = scp.tile([128, 2, H, TX], f16, name="sc2gh", tag="sc2gh")
            sc2s = [scp.tile([128, 8, H, TX], f16, name=f"sc2{b}", tag=f"sc2{b}") for b in range(2)]

            # Row halves: half A covers V rows 0..29 (x rows 0..28),
            # half B V rows 30..57 (x rows 29..55).
            halves = [(0, 30, 0, 29), (30, VR, 29, H)]

            def emit_x_dma(img):
                pg = img % 2
                for icc in range(2):
                    xr = xrs[icc][pg]
                    for (_, _, xa, xb) in halves:
                        nc.sync.dma_start(
                            out=xr[:, xa:xb, :],
                            in_=x_d[img, icc * 128 : (icc + 1) * 128, xa:xb, :],
                        )

            def emit_split_scale(img, icc, split_halves):
                """ACT: phase-split + prescaled planes for one (img, icc)."""
                pg = img % 2
                xr, ph = xrs[icc][pg], phs[icc][pg]
                hs = split_halves if split_halves is not None else [(0, VR, 0, H)]
                for (va, vb, xa, xb) in hs:
                    # phase p holds padded col c = 4*blk + p; data col w = c-1.
                    r0 = va + 1 if va == 0 else va
                    r1 = vb if vb != VR else VR - 1
                    nc.scalar.copy(out=ph[:, 0, r0:r1, 1:PHB], in_=xr[:, xa:xb, 3:H:4])
                    nc.scalar.copy(out=ph[:, 1, r0:r1, 0:14], in_=xr[:, xa:xb, 0:H:4])
                    nc.scalar.copy(out=ph[:, 2, r0:r1, 0:14], in_=xr[:, xa:xb, 1:H:4])
                    nc.scalar.copy(out=ph[:, 3, r0:r1, 0:14], in_=xr[:, xa:xb, 2:H:4])
                    nc.scalar.mul(ph[:, 4, r0:r1, 0:14], ph[:, 1, r0:r1, 0:14], 4.0)
                    nc.scalar.mul(ph[:, 5, r0:r1, 0:14], ph[:, 2, r0:r1, 0:14], 4.0)

            def emit_v_ops(icc, img, va, vb):
                """V-transform rows va..vb for one (icc, img).  DVE does the
                dependency-bearing work; GpSimd computes only leaves (E2s ->
                V3/V4) that feed the PE, never the DVE."""
                pg = img % 2
                ph, vt, sc_d = phs[icc][pg], vts[icc][pg], sc_ds[pg]
                q0 = ph[:, 0, va:vb, 0:TX]
                q1 = ph[:, 1, va:vb, 0:TX]
                q2 = ph[:, 2, va:vb, 0:TX]
                q3 = ph[:, 3, va:vb, 0:TX]
                q4 = ph[:, 0, va:vb, 1:PHB]
                q5 = ph[:, 1, va:vb, 1:PHB]
                b4 = ph[:, 4, va:vb, 0:TX]      # 4*E1[tx]
                d4 = ph[:, 5, va:vb, 0:TX]      # 4*E2[tx]
                B = sc_d[:, 0, va:vb, :]
                D = sc_d[:, 1, va:vb, :]
                Es = sc_d[:, 2, va:vb, :]
                F = sc_d[:, 3, va:vb, :]
                w5 = sc_d[:, 4, va:vb, :]
                s = sc_d[:, 5, va:vb, :]
                t = sc_d[:, 6, va:vb, :]
                u = sc_d[:, 7, va:vb, :]
                E2s = sc_g[:, 0, va:vb, :]
                v = lambda m: vt[:, m, va:vb, :]
                nc.vector.tensor_tensor(Es, q1, q3, SUB)
                nc.vector.tensor_tensor(F, q4, q2, SUB)
                nc.gpsimd.tensor_tensor(E2s, Es, Es, ADD)     # leaf chain ->
                nc.gpsimd.tensor_tensor(v(3), F, E2s, SUB)    # PE only
                nc.gpsimd.tensor_tensor(v(4), F, E2s, ADD)
                nc.vector.tensor_tensor(B, q3, q4, ADD)
                nc.vector.tensor_tensor(D, q3, q4, SUB)
                nc.vector.tensor_tensor(w5, q5, q3, SUB)
                nc.vector.tensor_tensor(s, b4, d4, ADD)
                nc.vector.tensor_tensor(t, b4, d4, SUB)
                nc.vector.tensor_tensor(u, q0, q2, SUB)
                nc.vector.scalar_tensor_tensor(v(0), u, 4.0, F, MUL, ADD)  # 4q0-5q2+q4
                nc.vector.tensor_tensor(v(1), B, s, SUB)      # -4q1-4q2+q3+q4
                nc.vector.tensor_tensor(v(2), t, D, SUB)      # 4q1-4q2-q3+q4
                nc.vector.scalar_tensor_tensor(v(5), Es, 4.0, w5, MUL, ADD)

            def emit_v(img, halved=False):
                if halved:
                    for (va, vb, _, _) in halves:
                        for icc in range(2):
                            emit_v_ops(icc, img, va, vb)
                else:
                    for icc in range(2):
                        emit_v_ops(icc, img, 0, VR)

            def emit_compute_stage(img, act_inject=()):
                """GEMMs + evictions + output transform for one image.
                act_inject: list of (after_group_idx, fn) callbacks that emit
                extra ACT work between eviction groups so the ACT stream stays
                interleaved (splits for img+2 must not queue behind a full
                image of evictions)."""
                pg = img % 2
                inject = dict()
                for g, fn in act_inject:
                    inject.setdefault(g, []).append(fn)
                gidx = 0
                for occ in range(2):
                    msb = msbs[(img * 2 + occ) % 2]
                    for chunk in range(2):
                        c0 = chunk * 28
                        for m in range(6):
                            mt = pp.tile([128, 28, TX], f32, name="mt", tag="mt")
                            t = 0
                            for ky in range(3):
                                for icc in range(2):
                                    widx = ((occ * 6 + m) * 3 + ky) * 2 + icc
                                    nc.tensor.matmul(
                                        mt[:],
                                        w_sb[:, widx * 128 : (widx + 1) * 128],
                                        vts[icc][pg][:, m, c0 + ky : c0 + ky + 28, :],
                                        start=(t == 0),
                                        stop=(t == 5),
                                    )
                                    t += 1
                            nc.scalar.copy(out=msb[:, m, c0 : c0 + 28, :], in_=mt[:])
                        gidx += 1
                        for fn in inject.get(gidx, []):
                            fn()
                    # output transform for this occ on whole [56,14] slabs.
                    ms = lambda m: msb[:, m, :, :]
                    s2 = sc2s[pg]
                    I_ = s2[:, 0, :, :]
                    J_ = s2[:, 1, :, :]
                    G2 = sc2gh[:, 0, :, :]
                    H2 = sc2gh[:, 1, :, :]
                    y3t = s2[:, 2, :, :]
                    IJ = s2[:, 3, :, :]
                    yv = [s2[:, 4 + v, :, :] for v in range(4)]
                    nc.vector.tensor_tensor(I_, ms(1), ms(2), ADD)
                    nc.vector.tensor_tensor(J_, ms(3), ms(4), ADD)
                    nc.vector.tensor_tensor(G2, ms(1), ms(2), SUB)
                    nc.vector.tensor_tensor(H2, ms(3), ms(4), SUB)
                    nc.vector.scalar_tensor_tensor(y3t, H2, 8.0, G2, MUL, ADD)
                    nc.vector.scalar_tensor_tensor(yv[1], H2, 2.0, G2, MUL, ADD)
                    nc.vector.scalar_tensor_tensor(yv[2], J_, 4.0, I_, MUL, ADD)
                    nc.gpsimd.tensor_tensor(IJ, I_, J_, ADD)
                    nc.gpsimd.tensor_tensor(yv[0], IJ, ms(0), ADD)
                    nc.gpsimd.tensor_tensor(yv[3], y3t, ms(5), ADD)
                    for v_i in range(4):
                        nc.sync.dma_start(
                            out=y_d[img, occ * 128 : (occ + 1) * 128, v_i, :, :],
                            in_=yv[v_i],
                        )

            # ---- software-pipelined emission ----
            emit_x_dma(0)
            emit_x_dma(1)
            # interleave img0's splits by half so V (and the first GEMMs)
            # start as soon as half A of both iccs is ready
            for hv in halves:
                for icc in range(2):
                    emit_split_scale(0, icc, [hv])
            for icc in range(2):
                emit_split_scale(1, icc, None)
            emit_v(0, halved=True)
            emit_x_dma(2)
            emit_v(1)
            emit_compute_stage(
                0,
                act_inject=[
                    (1, lambda: emit_split_scale(2, 0, None)),
                    (2, lambda: emit_split_scale(2, 1, None)),
                ],
            )
            emit_x_dma(3)
            emit_v(2)
            emit_compute_stage(
                1,
                act_inject=[
                    (1, lambda: emit_split_scale(3, 0, None)),
                    (2, lambda: emit_split_scale(3, 1, None)),
                ],
            )
            emit_v(3)
            emit_compute_stage(2)
            emit_compute_stage(3)

    _split_waits(nc)
    return nc


def _prep_weight(weight: np.ndarray, mask: np.ndarray) -> np.ndarray:
    """[OC, IC, K, K] masked weight -> Winograd-transformed lhsT tiles
    [128ic, (occ,m,ky,icc)*128oc]."""
    G = np.array(
        [
            [1 / 4, 0, 0],
            [-1 / 6, -1 / 6, -1 / 6],
            [-1 / 6, 1 / 6, -1 / 6],
            [1 / 24, 1 / 12, 1 / 6],
            [1 / 24, -1 / 12, 1 / 6],
            [0, 0, 1],
        ],
        np.float32,
    )
    wm = (weight * mask).astype(np.float32)                  # [oc, ic, ky, kx]
    wp = np.einsum("mx,oikx->moik", G, wm)                   # [m, oc, ic, ky]
    t = wp.reshape(6, 2, 128, 2, 128, 3)                     # [m, occ, oc, icc, ic, ky]
    t = t.transpose(4, 1, 0, 5, 3, 2)                        # [ic, occ, m, ky, icc, oc]
    return np.ascontiguousarray(t.reshape(128, NW * 128).astype(np.float16))


def kernel(x: np.ndarray, weight: np.ndarray, mask: np.ndarray) -> np.ndarray:
    from concourse.bass_utils import run_bass_kernel_spmd

    x = np.asarray(x, dtype=np.float32)
    x16 = np.ascontiguousarray(x.astype(np.float16))
    w_host = _prep_weight(np.asarray(weight), np.asarray(mask))

    nc = _build_nc()
    in_maps = [
        {
            "x": np.ascontiguousarray(x16[c * IMG_PER_CORE : (c + 1) * IMG_PER_CORE]),
            "w": w_host,
        }
        for c in range(N_CORES)
    ]
    res = run_bass_kernel_spmd(nc, in_maps, core_ids=list(range(N_CORES)))
    out = np.empty_like(x)
    for c in range(N_CORES):
        yp = res.results[c]["y"]  # [4, C, 4, 56, 14] phase-planar fp16
        yi = np.transpose(yp, (0, 1, 3, 4, 2)).reshape(IMG_PER_CORE, C, H, H)
        out[c * IMG_PER_CORE : (c + 1) * IMG_PER_CORE] = yi.astype(np.float32)
    return out


# revision 21
# speedup vs baseline: 1.0061x; 1.0061x over previous
"""ExpanderConv2d as a Bass/Tile kernel for Trainium2, data-parallel over batch
across 8 NeuronCores.

Reference op: y = conv2d(x, weight * mask), N=32, C=256->256, 56x56, k=3,
stride 1, pad 1.

v5: 1D Winograd F(4,3) along W.  Each quad of 4 horizontal outputs costs 6
multiplies instead of 12, so the PE streams 2/3 the columns of the direct
9-tap formulation: 226k columns/core (~94us at 1 col/cycle) vs 451k (~188us).

Per (icc, img): x [128,56,56] fp16 -> ACT phase-split into 4 column-phase
planes [58 rows, 15 blocks] plus 3 ACT-prescaled planes (4*E0, 4*E1, 4*E2;
exact in fp16) -> V[m] for the 6 Winograd components, mostly as plain
tensor_tensor at the DVE 2x rate (the prescaled planes remove the slow 1x
scalar_tensor_tensor ops) -> GEMMs M[m] = sum_{ky,icc} W'[m,ky,icc]^T V[m]
(PSUM fp32, moving dim = 28 rows x 14 tiles = 392) -> ACT evicts M to SBUF
fp16 -> output transform y[v] = At M on [56,14] slabs -> fp16 phase-planar
DMA; the host interleaves the 4 phases (pure transpose) and upcasts.

Engine discipline learned from traces: GpSimd ops are ~3x slower than DVE
and must never produce values DVE waits on (only leaf work: V3/V4, IJ/yv0/
yv3); the ACT stream is hand-interleaved so M evictions track the PE's
PSUM-bank rotation with next-image phase splits injected in the gaps.

Sharding: batch 32 -> 4 images per core; the transformed masked weight
(2.4 MB fp16, 72 [128x128] tiles) is replicated to every core.
"""

import numpy as np

N_CORES = 8
IMG_PER_CORE = 4
C = 256
H = 56
TX = 14          # winograd tiles per row (4 outputs each)
PHB = 15         # phase-plane blocks (E0/E1 need 15, E2/E3 use 14)
VR = 58          # V rows = padded rows
NW = 72          # weight tiles: occ(2) x m(6) x ky(3) x icc(2)


def _split_waits(nc, max_waits=1):
    """walrus in this container rejects instructions carrying more than one
    semaphore wait ("Too many sync wait commands").  Hoist the extra waits onto
    injected single-wait NoOps on the same engine just before the instruction —
    sem waits block the engine, so a chain of single waits is equivalent."""
    import concourse.mybir as mybir

    for f in nc.m.functions:
        for blk in f.blocks:
            out = []
            changed = False
            for inst in blk.instructions:
                si = inst.sync_info
                if si and si.on_wait and len(si.on_wait) > max_waits:
                    waits = list(si.on_wait)
                    extra, keep = waits[:-max_waits], waits[-max_waits:]
                    for j, w in enumerate(extra):
                        out.append(
                            mybir.InstNoOp(
                                name=f"{inst.name}-w{j}",
                                engine=inst.engine,
                                ins=[],
                                outs=[],
                                sync_info=mybir.SyncInfo(on_wait=[w], on_update=[]),
                                bass_nofuse=True,
                            )
                        )
                    si.on_wait = keep
                    changed = True
                out.append(inst)
            if changed:
                blk.instructions = out


def _build_nc():
    import concourse.bass as bass
    import concourse.mybir as mybir
    from concourse.tile import TileContext

    f32 = mybir.dt.float32
    f16 = mybir.dt.float16
    ADD = mybir.AluOpType.add
    SUB = mybir.AluOpType.subtract
    MUL = mybir.AluOpType.mult

    nc = bass.Bass("TRN2", target_bir_lowering=False, debug=False)
    x_d = nc.dram_tensor("x", [IMG_PER_CORE, C, H, H], f16, kind="ExternalInput").ap()
    w_d = nc.dram_tensor("w", [128, NW * 128], f16, kind="ExternalInput").ap()
    # y is stored phase-planar: y[img, c, v, h, tx] = out[img, c, h, 4*tx+v]
    y_d = nc.dram_tensor("y", [IMG_PER_CORE, C, 4, H, TX], f16, kind="ExternalOutput").ap()

    with TileContext(nc) as tc:
        with (
            tc.tile_pool(name="wpool", bufs=1) as wp,
            tc.tile_pool(name="xpool", bufs=1) as xp,
            tc.tile_pool(name="psum", bufs=8, space="PSUM") as pp,
            tc.tile_pool(name="msb", bufs=1) as mp,
            tc.tile_pool(name="scp", bufs=1) as scp,
        ):
            w_sb = wp.tile([128, NW * 128], f16, name="w_sb", tag="w_sb")
            # Each GEMM group consumes 6 consecutive tiles ((occ,m) x ky x icc);
            # DMA in matching chunks so groups start as their tiles land.
            wq = 0
            for wn in (6, 6, 6, 6, 6, 6, 18, 18):
                nc.scalar.dma_start(
                    out=w_sb[:, wq * 128 : (wq + wn) * 128],
                    in_=w_d[:, wq * 128 : (wq + wn) * 128],
                )
                wq += wn

            # Warm the PE clock gate (HAM) with throwaway matmuls on scratch
            # data while the first input/weight DMAs are in flight.
            warm = wp.tile([128, 392], f16, name="warm", tag="warm")
            nc.vector.memset(warm[:], 0.0)
            for i in range(10):
                warm_ps = pp.tile([128, 28, TX], f32, name="mt", tag="mt")
                nc.tensor.matmul(
                    warm_ps[:], warm[:, :128], warm[:, :392], start=True, stop=True
                )

            # Per (icc, ping/pong): raw input, phase planes (4 raw + 3
            # prescaled), V components.
            xrs = [
                [xp.tile([128, H, H], f16, name=f"xr{i}{b}", tag=f"xr{i}{b}") for b in range(2)]
                for i in range(2)
            ]
            phs = [
                [xp.tile([128, 6, VR, PHB], f16, name=f"ph{i}{b}", tag=f"ph{i}{b}") for b in range(2)]
                for i in range(2)
            ]
            vts = [
                [xp.tile([128, 6, VR, TX], f16, name=f"vt{i}{b}", tag=f"vt{i}{b}") for b in range(2)]
                for i in range(2)
            ]
            # Zero the padding cells of the phase planes once; DMA/split/scale
            # only ever write interior rows/blocks, so they stay zero.
            for i in range(2):
                for b in range(2):
                    ph = phs[i][b]
                    nc.gpsimd.memset(ph[:, :, 0, :], 0.0)        # top pad row
                    nc.gpsimd.memset(ph[:, :, VR - 1, :], 0.0)   # bottom pad row
                    nc.gpsimd.memset(ph[:, 0, :, 0], 0.0)        # left pad col (E0 blk 0)
                    nc.gpsimd.memset(ph[:, 1, :, PHB - 1], 0.0)  # right pad col (E1 blk 14)

            # V scratch, ping-ponged per image: a single shared buffer makes
            # DVE's img N+1 subexpression writes wait on GpSimd's img N reads
            # (observed 10us cross-engine WAR stalls).
            # sc_d slabs (DVE-private): 0:B 1:D 2:w5 3:s 4:t 5:u
            sc_ds = [scp.tile([128, 6, VR, TX], f16, name=f"sc_d{b}", tag=f"sc_d{b}") for b in range(2)]
            # Es/F are read by GpSimd (E2s -> V3/V4); keep a copy per
            # (icc, ping-pong) so DVE's next block never WAR-waits on the
            # slower engine.
            sc_efs = [
                [scp.tile([128, 2, VR, TX], f16, name=f"ef{i}{b}", tag=f"ef{i}{b}") for b in range(2)]
                for i in range(2)
            ]
            sc_g = scp.tile([128, 1, VR, TX], f16, name="sc_g", tag="sc_g")
            msbs = [mp.tile([128, 6, H, TX], f16, name=f"m{b}", tag=f"m{b}") for b in range(2)]
            # out-transform scratch: G2/H2 stay DVE-private (single buffer);
            # slabs read by GpSimd or DMA'd (I,J,y3t,IJ,yv0-3) ping-pong.
            sc2gh = scp.tile([128, 2, H, TX], f16, name="sc2gh", tag="sc2gh")
            sc2s = [scp.tile([128, 8, H, TX], f16, name=f"sc2{b}", tag=f"sc2{b}") for b in range(2)]

            # Row halves: half A covers V rows 0..29 (x rows 0..28),
            # half B V rows 30..57 (x rows 29..55).
            halves = [(0, 30, 0, 29), (30, VR, 29, H)]

            def emit_x_dma(img):
                pg = img % 2
                for icc in range(2):
                    xr = xrs[icc][pg]
                    for (_, _, xa, xb) in halves:
                        nc.sync.dma_start(
                            out=xr[:, xa:xb, :],
                            in_=x_d[img, icc * 128 : (icc + 1) * 128, xa:xb, :],
                        )

            def emit_split_scale(img, icc, split_halves):
                """ACT: phase-split + prescaled planes for one (img, icc)."""
                pg = img % 2
                xr, ph = xrs[icc][pg], phs[icc][pg]
                hs = split_halves if split_halves is not None else [(0, VR, 0, H)]
                for (va, vb, xa, xb) in hs:
                    # phase p holds padded col c = 4*blk + p; data col w = c-1.
                    r0 = va + 1 if va == 0 else va
                    r1 = vb if vb != VR else VR - 1
                    nc.scalar.copy(out=ph[:, 0, r0:r1, 1:PHB], in_=xr[:, xa:xb, 3:H:4])
                    nc.scalar.copy(out=ph[:, 1, r0:r1, 0:14], in_=xr[:, xa:xb, 0:H:4])
                    nc.scalar.copy(out=ph[:, 2, r0:r1, 0:14], in_=xr[:, xa:xb, 1:H:4])
                    nc.scalar.copy(out=ph[:, 3, r0:r1, 0:14], in_=xr[:, xa:xb, 2:H:4])
                    nc.scalar.mul(ph[:, 4, r0:r1, 0:14], ph[:, 1, r0:r1, 0:14], 4.0)
                    nc.scalar.mul(ph[:, 5, r0:r1, 0:14], ph[:, 2, r0:r1, 0:14], 4.0)

            def emit_v_ops(icc, img, va, vb):
                """V-transform rows va..vb for one (icc, img).  DVE does the
                dependency-bearing work; GpSimd computes only leaves (E2s ->
                V3/V4) that feed the PE, never the DVE."""
                pg = img % 2
                ph, vt, sc_d = phs[icc][pg], vts[icc][pg], sc_ds[pg]
                q0 = ph[:, 0, va:vb, 0:TX]
                q1 = ph[:, 1, va:vb, 0:TX]
                q2 = ph[:, 2, va:vb, 0:TX]
                q3 = ph[:, 3, va:vb, 0:TX]
                q4 = ph[:, 0, va:vb, 1:PHB]
                q5 = ph[:, 1, va:vb, 1:PHB]
                b4 = ph[:, 4, va:vb, 0:TX]      # 4*E1[tx]
                d4 = ph[:, 5, va:vb, 0:TX]      # 4*E2[tx]
                B = sc_d[:, 0, va:vb, :]
                D = sc_d[:, 1, va:vb, :]
                w5 = sc_d[:, 2, va:vb, :]
                s = sc_d[:, 3, va:vb, :]
                t = sc_d[:, 4, va:vb, :]
                u = sc_d[:, 5, va:vb, :]
                Es = sc_efs[icc][pg][:, 0, va:vb, :]
                F = sc_efs[icc][pg][:, 1, va:vb, :]
                E2s = sc_g[:, 0, va:vb, :]
                v = lambda m: vt[:, m, va:vb, :]
                nc.vector.tensor_tensor(Es, q1, q3, SUB)
                nc.vector.tensor_tensor(F, q4, q2, SUB)
                nc.gpsimd.tensor_tensor(E2s, Es, Es, ADD)     # leaf chain ->
                nc.gpsimd.tensor_tensor(v(3), F, E2s, SUB)    # PE only
                nc.gpsimd.tensor_tensor(v(4), F, E2s, ADD)
                nc.vector.tensor_tensor(B, q3, q4, ADD)
                nc.vector.tensor_tensor(D, q3, q4, SUB)
                nc.vector.tensor_tensor(w5, q5, q3, SUB)
                nc.vector.tensor_tensor(s, b4, d4, ADD)
                nc.vector.tensor_tensor(t, b4, d4, SUB)
                nc.vector.tensor_tensor(u, q0, q2, SUB)
                nc.vector.scalar_tensor_tensor(v(0), u, 4.0, F, MUL, ADD)  # 4q0-5q2+q4
                nc.vector.tensor_tensor(v(1), B, s, SUB)      # -4q1-4q2+q3+q4
                nc.vector.tensor_tensor(v(2), t, D, SUB)      # 4q1-4q2-q3+q4
                nc.vector.scalar_tensor_tensor(v(5), Es, 4.0, w5, MUL, ADD)

            def emit_v(img, halved=False):
                if halved:
                    for (va, vb, _, _) in halves:
                        for icc in range(2):
                            emit_v_ops(icc, img, va, vb)
                else:
                    for icc in range(2):
                        emit_v_ops(icc, img, 0, VR)

            def emit_compute_stage(img, act_inject=()):
                """GEMMs + evictions + output transform for one image.
                act_inject: list of (after_group_idx, fn) callbacks that emit
                extra ACT work between eviction groups so the ACT stream stays
                interleaved (splits for img+2 must not queue behind a full
                image of evictions)."""
                pg = img % 2
                inject = dict()
                for g, fn in act_inject:
                    inject.setdefault(g, []).append(fn)
                gidx = 0
                for occ in range(2):
                    msb = msbs[(img * 2 + occ) % 2]
                    for chunk in range(2):
                        c0 = chunk * 28
                        for m in range(6):
                            mt = pp.tile([128, 28, TX], f32, name="mt", tag="mt")
                            t = 0
                            for ky in range(3):
                                for icc in range(2):
                                    widx = ((occ * 6 + m) * 3 + ky) * 2 + icc
                                    nc.tensor.matmul(
                                        mt[:],
                                        w_sb[:, widx * 128 : (widx + 1) * 128],
                                        vts[icc][pg][:, m, c0 + ky : c0 + ky + 28, :],
                                        start=(t == 0),
                                        stop=(t == 5),
                                    )
                                    t += 1
                            nc.scalar.copy(out=msb[:, m, c0 : c0 + 28, :], in_=mt[:])
                        gidx += 1
                        for fn in inject.get(gidx, []):
                            fn()
                    # output transform for this occ on whole [56,14] slabs.
                    ms = lambda m: msb[:, m, :, :]
                    s2 = sc2s[pg]
                    I_ = s2[:, 0, :, :]
                    J_ = s2[:, 1, :, :]
                    G2 = sc2gh[:, 0, :, :]
                    H2 = sc2gh[:, 1, :, :]
                    y3t = s2[:, 2, :, :]
                    IJ = s2[:, 3, :, :]
                    yv = [s2[:, 4 + v, :, :] for v in range(4)]
                    nc.vector.tensor_tensor(I_, ms(1), ms(2), ADD)
                    nc.vector.tensor_tensor(J_, ms(3), ms(4), ADD)
                    nc.vector.tensor_tensor(G2, ms(1), ms(2), SUB)
                    nc.vector.tensor_tensor(H2, ms(3), ms(4), SUB)
                    nc.vector.scalar_tensor_tensor(y3t, H2, 8.0, G2, MUL, ADD)
                    nc.vector.scalar_tensor_tensor(yv[1], H2, 2.0, G2, MUL, ADD)
                    nc.vector.scalar_tensor_tensor(yv[2], J_, 4.0, I_, MUL, ADD)
                    nc.gpsimd.tensor_tensor(IJ, I_, J_, ADD)
                    nc.gpsimd.tensor_tensor(yv[0], IJ, ms(0), ADD)
                    nc.gpsimd.tensor_tensor(yv[3], y3t, ms(5), ADD)
                    for v_i in range(4):
                        nc.sync.dma_start(
                            out=y_d[img, occ * 128 : (occ + 1) * 128, v_i, :, :],
                            in_=yv[v_i],
                        )

            # ---- software-pipelined emission ----
            emit_x_dma(0)
            emit_x_dma(1)
            # interleave img0's splits by half so V (and the first GEMMs)
            # start as soon as half A of both iccs is ready
            for hv in halves:
                for icc in range(2):
                    emit_split_scale(0, icc, [hv])
            for icc in range(2):
                emit_split_scale(1, icc, None)
            emit_v(0, halved=True)
            emit_x_dma(2)
            emit_v(1)
            emit_compute_stage(
                0,
                act_inject=[
                    (1, lambda: emit_split_scale(2, 0, None)),
                    (2, lambda: emit_split_scale(2, 1, None)),
                ],
            )
            emit_x_dma(3)
            emit_v(2)
            emit_compute_stage(
                1,
                act_inject=[
                    (1, lambda: emit_split_scale(3, 0, None)),
                    (2, lambda: emit_split_scale(3, 1, None)),
                ],
            )
            emit_v(3)
            emit_compute_stage(2)
            emit_compute_stage(3)

    _split_waits(nc)
    return nc


def _prep_weight(weight: np.ndarray, mask: np.ndarray) -> np.ndarray:
    """[OC, IC, K, K] masked weight -> Winograd-transformed lhsT tiles
    [128ic, (occ,m,ky,icc)*128oc]."""
    G = np.array(
        [
            [1 / 4, 0, 0],
            [-1 / 6, -1 / 6, -1 / 6],
            [-1 / 6, 1 / 6, -1 / 6],
            [1 / 24, 1 / 12, 1 / 6],
            [1 / 24, -1 / 12, 1 / 6],
            [0, 0, 1],
        ],
        np.float32,
    )
    wm = (weight * mask).astype(np.float32)                  # [oc, ic, ky, kx]
    wp = np.einsum("mx,oikx->moik", G, wm)                   # [m, oc, ic, ky]
    t = wp.reshape(6, 2, 128, 2, 128, 3)                     # [m, occ, oc, icc, ic, ky]
    t = t.transpose(4, 1, 0, 5, 3, 2)                        # [ic, occ, m, ky, icc, oc]
    return np.ascontiguousarray(t.reshape(128, NW * 128).astype(np.float16))


def kernel(x: np.ndarray, weight: np.ndarray, mask: np.ndarray) -> np.ndarray:
    from concourse.bass_utils import run_bass_kernel_spmd

    x = np.asarray(x, dtype=np.float32)
    x16 = np.ascontiguousarray(x.astype(np.float16))
    w_host = _prep_weight(np.asarray(weight), np.asarray(mask))

    nc = _build_nc()
    in_maps = [
        {
            "x": np.ascontiguousarray(x16[c * IMG_PER_CORE : (c + 1) * IMG_PER_CORE]),
            "w": w_host,
        }
        for c in range(N_CORES)
    ]
    res = run_bass_kernel_spmd(nc, in_maps, core_ids=list(range(N_CORES)))
    out = np.empty_like(x)
    for c in range(N_CORES):
        yp = res.results[c]["y"]  # [4, C, 4, 56, 14] phase-planar fp16
        yi = np.transpose(yp, (0, 1, 3, 4, 2)).reshape(IMG_PER_CORE, C, H, H)
        out[c * IMG_PER_CORE : (c + 1) * IMG_PER_CORE] = yi.astype(np.float32)
    return out


# revision 24
# speedup vs baseline: 1.0683x; 1.0618x over previous
"""ExpanderConv2d as a Bass/Tile kernel for Trainium2, data-parallel over batch
across 8 NeuronCores.

Reference op: y = conv2d(x, weight * mask), N=32, C=256->256, 56x56, k=3,
stride 1, pad 1.

v5: 1D Winograd F(4,3) along W.  Each quad of 4 horizontal outputs costs 6
multiplies instead of 12, so the PE streams 2/3 the columns of the direct
9-tap formulation: 226k columns/core (~94us at 1 col/cycle) vs 451k (~188us).

Per (icc, img): x [128,56,56] fp16 -> ACT phase-split into 4 column-phase
planes [58 rows, 15 blocks] plus 3 ACT-prescaled planes (4*E0, 4*E1, 4*E2;
exact in fp16) -> V[m] for the 6 Winograd components, mostly as plain
tensor_tensor at the DVE 2x rate (the prescaled planes remove the slow 1x
scalar_tensor_tensor ops) -> GEMMs M[m] = sum_{ky,icc} W'[m,ky,icc]^T V[m]
(PSUM fp32, moving dim = 28 rows x 14 tiles = 392) -> ACT evicts M to SBUF
fp16 -> output transform y[v] = At M on [56,14] slabs -> fp16 phase-planar
DMA; the host interleaves the 4 phases (pure transpose) and upcasts.

Engine discipline learned from traces: GpSimd ops are ~3x slower than DVE
and must never produce values DVE waits on (only leaf work: V3/V4, IJ/yv0/
yv3); the ACT stream is hand-interleaved so M evictions track the PE's
PSUM-bank rotation with next-image phase splits injected in the gaps.

Sharding: batch 32 -> 4 images per core; the transformed masked weight
(2.4 MB fp16, 72 [128x128] tiles) is replicated to every core.
"""

import numpy as np

N_CORES = 8
IMG_PER_CORE = 4
C = 256
H = 56
TX = 14          # winograd tiles per row (4 outputs each)
PHB = 15         # phase-plane blocks (E0/E1 need 15, E2/E3 use 14)
VR = 58          # V rows = padded rows
NW = 72          # weight tiles: occ(2) x m(6) x ky(3) x icc(2)


def _split_waits(nc, max_waits=1):
    """walrus in this container rejects instructions carrying more than one
    semaphore wait ("Too many sync wait commands").  Hoist the extra waits onto
    injected single-wait NoOps on the same engine just before the instruction —
    sem waits block the engine, so a chain of single waits is equivalent."""
    import concourse.mybir as mybir

    for f in nc.m.functions:
        for blk in f.blocks:
            out = []
            changed = False
            for inst in blk.instructions:
                si = inst.sync_info
                if si and si.on_wait and len(si.on_wait) > max_waits:
                    waits = list(si.on_wait)
                    extra, keep = waits[:-max_waits], waits[-max_waits:]
                    for j, w in enumerate(extra):
                        out.append(
                            mybir.InstNoOp(
                                name=f"{inst.name}-w{j}",
                                engine=inst.engine,
                                ins=[],
                                outs=[],
                                sync_info=mybir.SyncInfo(on_wait=[w], on_update=[]),
                                bass_nofuse=True,
                            )
                        )
                    si.on_wait = keep
                    changed = True
                out.append(inst)
            if changed:
                blk.instructions = out


def _build_nc():
    import concourse.bass as bass
    import concourse.mybir as mybir
    from concourse.tile import TileContext

    f32 = mybir.dt.float32
    f16 = mybir.dt.float16
    ADD = mybir.AluOpType.add
    SUB = mybir.AluOpType.subtract
    MUL = mybir.AluOpType.mult

    nc = bass.Bass("TRN2", target_bir_lowering=False, debug=False)
    x_d = nc.dram_tensor("x", [IMG_PER_CORE, C, H, H], f16, kind="ExternalInput").ap()
    w_d = nc.dram_tensor("w", [128, NW * 128], f16, kind="ExternalInput").ap()
    # y is stored phase-planar: y[img, c, v, h, tx] = out[img, c, h, 4*tx+v]
    y_d = nc.dram_tensor("y", [IMG_PER_CORE, C, 4, H, TX], f16, kind="ExternalOutput").ap()

    with TileContext(nc) as tc:
        with (
            tc.tile_pool(name="wpool", bufs=1) as wp,
            tc.tile_pool(name="xpool", bufs=1) as xp,
            tc.tile_pool(name="psum", bufs=8, space="PSUM") as pp,
            tc.tile_pool(name="msb", bufs=1) as mp,
            tc.tile_pool(name="scp", bufs=1) as scp,
        ):
            w_sb = wp.tile([128, NW * 128], f16, name="w_sb", tag="w_sb")
            # Each GEMM group consumes 6 consecutive tiles ((occ,m) x ky x icc);
            # DMA in matching chunks so groups start as their tiles land.
            wq = 0
            for wn in (6, 6, 6, 6, 6, 6, 18, 18):
                nc.scalar.dma_start(
                    out=w_sb[:, wq * 128 : (wq + wn) * 128],
                    in_=w_d[:, wq * 128 : (wq + wn) * 128],
                )
                wq += wn

            # Warm the PE clock gate (HAM) with throwaway matmuls on scratch
            # data while the first input/weight DMAs are in flight.
            warm = wp.tile([128, 392], f16, name="warm", tag="warm")
            nc.vector.memset(warm[:], 0.0)
            for i in range(10):
                warm_ps = pp.tile([128, 28, TX], f32, name="mt", tag="mt")
                nc.tensor.matmul(
                    warm_ps[:], warm[:, :128], warm[:, :392], start=True, stop=True
                )

            # Per (icc, ping/pong): raw input, phase planes (4 raw + 3
            # prescaled), V components.
            xrs = [
                [xp.tile([128, H, H], f16, name=f"xr{i}{b}", tag=f"xr{i}{b}") for b in range(2)]
                for i in range(2)
            ]
            phs = [
                [xp.tile([128, 6, VR, PHB], f16, name=f"ph{i}{b}", tag=f"ph{i}{b}") for b in range(2)]
                for i in range(2)
            ]
            vts = [
                [xp.tile([128, 6, VR, TX], f16, name=f"vt{i}{b}", tag=f"vt{i}{b}") for b in range(2)]
                for i in range(2)
            ]
            # Zero the padding cells of the phase planes once; DMA/split/scale
            # only ever write interior rows/blocks, so they stay zero.
            for i in range(2):
                for b in range(2):
                    ph = phs[i][b]
                    nc.gpsimd.memset(ph[:, :, 0, :], 0.0)        # top pad row
                    nc.gpsimd.memset(ph[:, :, VR - 1, :], 0.0)   # bottom pad row
                    nc.gpsimd.memset(ph[:, 0, :, 0], 0.0)        # left pad col (E0 blk 0)
                    nc.gpsimd.memset(ph[:, 1, :, PHB - 1], 0.0)  # right pad col (E1 blk 14)

            # V scratch, DVE-private (GpSimd stays out of the V path
            # entirely: every cross-engine scratch hand-off we tried produced
            # multi-us WAR stalls), ping-ponged per image.
            # slabs: 0:Es 1:F 2:B 3:D 4:s 5:t 6:u(/t5a)
            sc_ds = [scp.tile([128, 7, VR, TX], f16, name=f"sc_d{b}", tag=f"sc_d{b}") for b in range(2)]
            msbs = [mp.tile([128, 6, H, TX], f16, name=f"m{b}", tag=f"m{b}") for b in range(2)]
            # out-transform scratch: H2 is DVE-private; everything GpSimd
            # reads or DMA touches (I,J,G2,H2d,y3t,yv0-3) ping-pongs.
            sc2gh = scp.tile([128, 1, H, TX], f16, name="sc2gh", tag="sc2gh")
            sc2s = [scp.tile([128, 9, H, TX], f16, name=f"sc2{b}", tag=f"sc2{b}") for b in range(2)]

            # Row halves: half A covers V rows 0..29 (x rows 0..28),
            # half B V rows 30..57 (x rows 29..55).
            halves = [(0, 30, 0, 29), (30, VR, 29, H)]

            def emit_x_dma(img):
                pg = img % 2
                for icc in range(2):
                    xr = xrs[icc][pg]
                    for (_, _, xa, xb) in halves:
                        nc.sync.dma_start(
                            out=xr[:, xa:xb, :],
                            in_=x_d[img, icc * 128 : (icc + 1) * 128, xa:xb, :],
                        )

            def emit_split_scale(img, icc, split_halves):
                """ACT: phase-split + prescaled planes for one (img, icc)."""
                pg = img % 2
                xr, ph = xrs[icc][pg], phs[icc][pg]
                hs = split_halves if split_halves is not None else [(0, VR, 0, H)]
                for (va, vb, xa, xb) in hs:
                    # phase p holds padded col c = 4*blk + p; data col w = c-1.
                    r0 = va + 1 if va == 0 else va
                    r1 = vb if vb != VR else VR - 1
                    nc.scalar.copy(out=ph[:, 0, r0:r1, 1:PHB], in_=xr[:, xa:xb, 3:H:4])
                    nc.scalar.copy(out=ph[:, 1, r0:r1, 0:14], in_=xr[:, xa:xb, 0:H:4])
                    nc.scalar.copy(out=ph[:, 2, r0:r1, 0:14], in_=xr[:, xa:xb, 1:H:4])
                    nc.scalar.copy(out=ph[:, 3, r0:r1, 0:14], in_=xr[:, xa:xb, 2:H:4])
                    nc.scalar.mul(ph[:, 4, r0:r1, 0:14], ph[:, 1, r0:r1, 0:14], 4.0)
                    nc.scalar.mul(ph[:, 5, r0:r1, 0:14], ph[:, 2, r0:r1, 0:14], 4.0)

            def emit_v_ops(icc, img, va, vb):
                """V-transform rows va..vb for one (icc, img).  DVE does the
                dependency-bearing work; GpSimd computes only leaves (E2s ->
                V3/V4) that feed the PE, never the DVE."""
                pg = img % 2
                ph, vt, sc_d = phs[icc][pg], vts[icc][pg], sc_ds[pg]
                q0 = ph[:, 0, va:vb, 0:TX]
                q1 = ph[:, 1, va:vb, 0:TX]
                q2 = ph[:, 2, va:vb, 0:TX]
                q3 = ph[:, 3, va:vb, 0:TX]
                q4 = ph[:, 0, va:vb, 1:PHB]
                q5 = ph[:, 1, va:vb, 1:PHB]
                b4 = ph[:, 4, va:vb, 0:TX]      # 4*E1[tx]
                d4 = ph[:, 5, va:vb, 0:TX]      # 4*E2[tx]
                Es = sc_d[:, 0, va:vb, :]
                F = sc_d[:, 1, va:vb, :]
                B = sc_d[:, 2, va:vb, :]
                D = sc_d[:, 3, va:vb, :]
                s = sc_d[:, 4, va:vb, :]
                t = sc_d[:, 5, va:vb, :]
                u = sc_d[:, 6, va:vb, :]
                v = lambda m: vt[:, m, va:vb, :]
                # emitted roughly in GEMM consumption order (m0, m1, ...)
                nc.vector.tensor_tensor(F, q4, q2, SUB)
                nc.vector.tensor_tensor(u, q0, q2, SUB)
                nc.vector.scalar_tensor_tensor(v(0), u, 4.0, F, MUL, ADD)  # 4q0-5q2+q4
                nc.vector.tensor_tensor(B, q3, q4, ADD)
                nc.vector.tensor_tensor(s, b4, d4, ADD)
                nc.vector.tensor_tensor(v(1), B, s, SUB)      # -4q1-4q2+q3+q4
                nc.vector.tensor_tensor(D, q3, q4, SUB)
                nc.vector.tensor_tensor(t, b4, d4, SUB)
                nc.vector.tensor_tensor(v(2), t, D, SUB)      # 4q1-4q2-q3+q4
                nc.vector.tensor_tensor(Es, q1, q3, SUB)
                nc.vector.scalar_tensor_tensor(v(3), Es, -2.0, F, MUL, ADD)
                nc.vector.scalar_tensor_tensor(v(4), Es, 2.0, F, MUL, ADD)
                nc.vector.scalar_tensor_tensor(u, Es, 4.0, q5, MUL, ADD)   # 4q1-4q3+q5
                nc.vector.tensor_tensor(v(5), u, q3, SUB)     # 4q1-5q3+q5

            def emit_v(img, halved=False):
                if halved:
                    for (va, vb, _, _) in halves:
                        for icc in range(2):
                            emit_v_ops(icc, img, va, vb)
                else:
                    for icc in range(2):
                        emit_v_ops(icc, img, 0, VR)

            def emit_compute_stage(img, act_inject=()):
                """GEMMs + evictions + output transform for one image.
                act_inject: list of (after_group_idx, fn) callbacks that emit
                extra ACT work between eviction groups so the ACT stream stays
                interleaved (splits for img+2 must not queue behind a full
                image of evictions)."""
                pg = img % 2
                inject = dict()
                for g, fn in act_inject:
                    inject.setdefault(g, []).append(fn)
                gidx = 0
                for occ in range(2):
                    msb = msbs[(img * 2 + occ) % 2]
                    for chunk in range(2):
                        c0 = chunk * 28
                        for m in range(6):
                            mt = pp.tile([128, 28, TX], f32, name="mt", tag="mt")
                            t = 0
                            for ky in range(3):
                                for icc in range(2):
                                    widx = ((occ * 6 + m) * 3 + ky) * 2 + icc
                                    nc.tensor.matmul(
                                        mt[:],
                                        w_sb[:, widx * 128 : (widx + 1) * 128],
                                        vts[icc][pg][:, m, c0 + ky : c0 + ky + 28, :],
                                        start=(t == 0),
                                        stop=(t == 5),
                                    )
                                    t += 1
                            nc.scalar.copy(out=msb[:, m, c0 : c0 + 28, :], in_=mt[:])
                        gidx += 1
                        for fn in inject.get(gidx, []):
                            fn()
                    # output transform for this occ on whole [56,14] slabs.
                    # DVE computes the msb-dependent intermediates promptly
                    # (releasing the msb buffer for the next image's evicts);
                    # GpSimd finishes the leaves.
                    ms = lambda m: msb[:, m, :, :]
                    s2 = sc2s[pg]
                    I_ = s2[:, 0, :, :]
                    J_ = s2[:, 1, :, :]
                    G2 = s2[:, 2, :, :]
                    H2d = s2[:, 3, :, :]
                    y3t = s2[:, 4, :, :]
                    IJ = s2[:, 5, :, :]
                    H2 = sc2gh[:, 0, :, :]
                    yv = [s2[:, 6, :, :], s2[:, 7, :, :], s2[:, 8, :, :], None]
                    nc.vector.tensor_tensor(I_, ms(1), ms(2), ADD)
                    nc.vector.tensor_tensor(J_, ms(3), ms(4), ADD)
                    nc.vector.tensor_tensor(G2, ms(1), ms(2), SUB)
                    nc.vector.tensor_tensor(H2, ms(3), ms(4), SUB)
                    nc.vector.tensor_tensor(H2d, H2, H2, ADD)
                    nc.vector.scalar_tensor_tensor(y3t, H2, 8.0, G2, MUL, ADD)
                    nc.vector.scalar_tensor_tensor(yv[2], J_, 4.0, I_, MUL, ADD)
                    nc.gpsimd.tensor_tensor(IJ, I_, J_, ADD)
                    nc.gpsimd.tensor_tensor(yv[0], IJ, ms(0), ADD)
                    nc.gpsimd.tensor_tensor(yv[1], G2, H2d, ADD)
                    yv3 = IJ  # reuse the IJ slab for the last phase
                    nc.gpsimd.tensor_tensor(yv3, y3t, ms(5), ADD)
                    yv[3] = yv3
                    for v_i in range(4):
                        nc.sync.dma_start(
                            out=y_d[img, occ * 128 : (occ + 1) * 128, v_i, :, :],
                            in_=yv[v_i],
                        )

            # ---- software-pipelined emission ----
            emit_x_dma(0)
            emit_x_dma(1)
            # interleave img0's splits by half so V (and the first GEMMs)
            # start as soon as half A of both iccs is ready
            for hv in halves:
                for icc in range(2):
                    emit_split_scale(0, icc, [hv])
            for icc in range(2):
                emit_split_scale(1, icc, None)
            emit_v(0, halved=True)
            emit_x_dma(2)
            emit_v(1)
            emit_compute_stage(
                0,
                act_inject=[
                    (1, lambda: emit_split_scale(2, 0, None)),
                    (2, lambda: emit_split_scale(2, 1, None)),
                ],
            )
            emit_x_dma(3)
            emit_v(2)
            emit_compute_stage(
                1,
                act_inject=[
                    (1, lambda: emit_split_scale(3, 0, None)),
                    (2, lambda: emit_split_scale(3, 1, None)),
                ],
            )
            emit_v(3)
            emit_compute_stage(2)
            emit_compute_stage(3)

    _split_waits(nc)
    return nc


def _prep_weight(weight: np.ndarray, mask: np.ndarray) -> np.ndarray:
    """[OC, IC, K, K] masked weight -> Winograd-transformed lhsT tiles
    [128ic, (occ,m,ky,icc)*128oc]."""
    G = np.array(
        [
            [1 / 4, 0, 0],
            [-1 / 6, -1 / 6, -1 / 6],
            [-1 / 6, 1 / 6, -1 / 6],
            [1 / 24, 1 / 12, 1 / 6],
            [1 / 24, -1 / 12, 1 / 6],
            [0, 0, 1],
        ],
        np.float32,
    )
    wm = (weight * mask).astype(np.float32)                  # [oc, ic, ky, kx]
    wp = np.einsum("mx,oikx->moik", G, wm)                   # [m, oc, ic, ky]
    t = wp.reshape(6, 2, 128, 2, 128, 3)                     # [m, occ, oc, icc, ic, ky]
    t = t.transpose(4, 1, 0, 5, 3, 2)                        # [ic, occ, m, ky, icc, oc]
    return np.ascontiguousarray(t.reshape(128, NW * 128).astype(np.float16))


def kernel(x: np.ndarray, weight: np.ndarray, mask: np.ndarray) -> np.ndarray:
    from concourse.bass_utils import run_bass_kernel_spmd

    x = np.asarray(x, dtype=np.float32)
    x16 = np.ascontiguousarray(x.astype(np.float16))
    w_host = _prep_weight(np.asarray(weight), np.asarray(mask))

    nc = _build_nc()
    in_maps = [
        {
            "x": np.ascontiguousarray(x16[c * IMG_PER_CORE : (c + 1) * IMG_PER_CORE]),
            "w": w_host,
        }
        for c in range(N_CORES)
    ]
    res = run_bass_kernel_spmd(nc, in_maps, core_ids=list(range(N_CORES)))
    out = np.empty_like(x)
    for c in range(N_CORES):
        yp = res.results[c]["y"]  # [4, C, 4, 56, 14] phase-planar fp16
        yi = np.transpose(yp, (0, 1, 3, 4, 2)).reshape(IMG_PER_CORE, C, H, H)
        out[c * IMG_PER_CORE : (c + 1) * IMG_PER_CORE] = yi.astype(np.float32)
    return out


# revision 27
# speedup vs baseline: 1.1133x; 1.0421x over previous
"""ExpanderConv2d as a Bass/Tile kernel for Trainium2, data-parallel over batch
across 8 NeuronCores.

Reference op: y = conv2d(x, weight * mask), N=32, C=256->256, 56x56, k=3,
stride 1, pad 1.

v5: 1D Winograd F(4,3) along W.  Each quad of 4 horizontal outputs costs 6
multiplies instead of 12, so the PE streams 2/3 the columns of the direct
9-tap formulation: 226k columns/core (~94us at 1 col/cycle) vs 451k (~188us).

Per (icc, img): x [128,56,56] fp16 -> ACT phase-split into 4 column-phase
planes [58 rows, 15 blocks] plus 3 ACT-prescaled planes (4*E0, 4*E1, 4*E2;
exact in fp16) -> V[m] for the 6 Winograd components, mostly as plain
tensor_tensor at the DVE 2x rate (the prescaled planes remove the slow 1x
scalar_tensor_tensor ops) -> GEMMs M[m] = sum_{ky,icc} W'[m,ky,icc]^T V[m]
(PSUM fp32, moving dim = 28 rows x 14 tiles = 392) -> ACT evicts M to SBUF
fp16 -> output transform y[v] = At M on [56,14] slabs -> fp16 phase-planar
DMA; the host interleaves the 4 phases (pure transpose) and upcasts.

Engine discipline learned from traces: GpSimd ops are ~3x slower than DVE
and must never produce values DVE waits on (only leaf work: V3/V4, IJ/yv0/
yv3); the ACT stream is hand-interleaved so M evictions track the PE's
PSUM-bank rotation with next-image phase splits injected in the gaps.

Sharding: batch 32 -> 4 images per core; the transformed masked weight
(2.4 MB fp16, 72 [128x128] tiles) is replicated to every core.
"""

import numpy as np

N_CORES = 8
IMG_PER_CORE = 4
C = 256
H = 56
TX = 14          # winograd tiles per row (4 outputs each)
PHB = 15         # phase-plane blocks (E0/E1 need 15, E2/E3 use 14)
VR = 58          # V rows = padded rows
NW = 72          # weight tiles: occ(2) x m(6) x ky(3) x icc(2)


def _split_waits(nc, max_waits=1):
    """walrus in this container rejects instructions carrying more than one
    semaphore wait ("Too many sync wait commands").  Hoist the extra waits onto
    injected single-wait NoOps on the same engine just before the instruction —
    sem waits block the engine, so a chain of single waits is equivalent."""
    import concourse.mybir as mybir

    for f in nc.m.functions:
        for blk in f.blocks:
            out = []
            changed = False
            for inst in blk.instructions:
                si = inst.sync_info
                if si and si.on_wait and len(si.on_wait) > max_waits:
                    waits = list(si.on_wait)
                    extra, keep = waits[:-max_waits], waits[-max_waits:]
                    for j, w in enumerate(extra):
                        out.append(
                            mybir.InstNoOp(
                                name=f"{inst.name}-w{j}",
                                engine=inst.engine,
                                ins=[],
                                outs=[],
                                sync_info=mybir.SyncInfo(on_wait=[w], on_update=[]),
                                bass_nofuse=True,
                            )
                        )
                    si.on_wait = keep
                    changed = True
                out.append(inst)
            if changed:
                blk.instructions = out


def _build_nc():
    import concourse.bass as bass
    import concourse.mybir as mybir
    from concourse.tile import TileContext

    f32 = mybir.dt.float32
    f16 = mybir.dt.float16
    ADD = mybir.AluOpType.add
    SUB = mybir.AluOpType.subtract
    MUL = mybir.AluOpType.mult

    nc = bass.Bass("TRN2", target_bir_lowering=False, debug=False)
    x_d = nc.dram_tensor("x", [IMG_PER_CORE, C, H, H], f16, kind="ExternalInput").ap()
    w_d = nc.dram_tensor("w", [128, NW * 128], f16, kind="ExternalInput").ap()
    # y is stored phase-planar: y[img, c, v, h, tx] = out[img, c, h, 4*tx+v]
    y_d = nc.dram_tensor("y", [IMG_PER_CORE, C, 4, H, TX], f16, kind="ExternalOutput").ap()

    with TileContext(nc) as tc:
        with (
            tc.tile_pool(name="wpool", bufs=1) as wp,
            tc.tile_pool(name="xpool", bufs=1) as xp,
            tc.tile_pool(name="psum", bufs=8, space="PSUM") as pp,
            tc.tile_pool(name="msb", bufs=1) as mp,
            tc.tile_pool(name="scp", bufs=1) as scp,
        ):
            w_sb = wp.tile([128, NW * 128], f16, name="w_sb", tag="w_sb")

            def emit_w_dma(wq, wn):
                nc.scalar.dma_start(
                    out=w_sb[:, wq * 128 : (wq + wn) * 128],
                    in_=w_d[:, wq * 128 : (wq + wn) * 128],
                )

            # Only the first GEMM group's 6 tiles go out ahead of x; the rest
            # interleave with the x DMAs below so the weight bulk doesn't hog
            # the DMA queues while img0's input is in flight.
            emit_w_dma(0, 6)

            # Warm the PE clock gate (HAM) with throwaway matmuls on scratch
            # data while the first input/weight DMAs are in flight.
            warm = wp.tile([128, 392], f16, name="warm", tag="warm")
            nc.vector.memset(warm[:], 0.0)
            for i in range(10):
                warm_ps = pp.tile([128, 28, TX], f32, name="mt", tag="mt")
                nc.tensor.matmul(
                    warm_ps[:], warm[:, :128], warm[:, :392], start=True, stop=True
                )

            # Per (icc, ping/pong): raw input, phase planes (4 raw + 3
            # prescaled), V components.
            xrs = [
                [xp.tile([128, H, H], f16, name=f"xr{i}{b}", tag=f"xr{i}{b}") for b in range(2)]
                for i in range(2)
            ]
            phs = [
                [xp.tile([128, 6, VR, PHB], f16, name=f"ph{i}{b}", tag=f"ph{i}{b}") for b in range(2)]
                for i in range(2)
            ]
            vts = [
                [xp.tile([128, 6, VR, TX], f16, name=f"vt{i}{b}", tag=f"vt{i}{b}") for b in range(2)]
                for i in range(2)
            ]
            # Zero the padding cells of the phase planes once; DMA/split/scale
            # only ever write interior rows/blocks, so they stay zero.
            for i in range(2):
                for b in range(2):
                    ph = phs[i][b]
                    nc.gpsimd.memset(ph[:, :, 0, :], 0.0)        # top pad row
                    nc.gpsimd.memset(ph[:, :, VR - 1, :], 0.0)   # bottom pad row
                    nc.gpsimd.memset(ph[:, 0, :, 0], 0.0)        # left pad col (E0 blk 0)
                    nc.gpsimd.memset(ph[:, 1, :, PHB - 1], 0.0)  # right pad col (E1 blk 14)

            # V scratch, DVE-private (GpSimd stays out of the V path
            # entirely: every cross-engine scratch hand-off we tried produced
            # multi-us WAR stalls), ping-ponged per image.
            # slabs: 0:Es 1:F 2:B 3:D 4:s 5:t 6:u(/t5a)
            sc_ds = [scp.tile([128, 7, VR, TX], f16, name=f"sc_d{b}", tag=f"sc_d{b}") for b in range(2)]
            msbs = [mp.tile([128, 6, H, TX], f16, name=f"m{b}", tag=f"m{b}") for b in range(2)]
            # out-transform scratch: H2 is DVE-private; everything GpSimd
            # reads or DMA touches (I,J,G2,H2d,y3t,yv0-3) ping-pongs.
            sc2gh = scp.tile([128, 1, H, TX], f16, name="sc2gh", tag="sc2gh")
            sc2s = [scp.tile([128, 9, H, TX], f16, name=f"sc2{b}", tag=f"sc2{b}") for b in range(2)]

            # Row halves: half A covers V rows 0..29 (x rows 0..28),
            # half B V rows 30..57 (x rows 29..55).
            halves = [(0, 30, 0, 29), (30, VR, 29, H)]

            def emit_x_dma(img):
                pg = img % 2
                for icc in range(2):
                    xr = xrs[icc][pg]
                    for (_, _, xa, xb) in halves:
                        nc.sync.dma_start(
                            out=xr[:, xa:xb, :],
                            in_=x_d[img, icc * 128 : (icc + 1) * 128, xa:xb, :],
                        )

            def emit_split_scale(img, icc, split_halves):
                """ACT: phase-split + prescaled planes for one (img, icc)."""
                pg = img % 2
                xr, ph = xrs[icc][pg], phs[icc][pg]
                hs = split_halves if split_halves is not None else [(0, VR, 0, H)]
                for (va, vb, xa, xb) in hs:
                    # phase p holds padded col c = 4*blk + p; data col w = c-1.
                    r0 = va + 1 if va == 0 else va
                    r1 = vb if vb != VR else VR - 1
                    nc.scalar.copy(out=ph[:, 0, r0:r1, 1:PHB], in_=xr[:, xa:xb, 3:H:4])
                    nc.scalar.copy(out=ph[:, 1, r0:r1, 0:14], in_=xr[:, xa:xb, 0:H:4])
                    nc.scalar.copy(out=ph[:, 2, r0:r1, 0:14], in_=xr[:, xa:xb, 1:H:4])
                    nc.scalar.copy(out=ph[:, 3, r0:r1, 0:14], in_=xr[:, xa:xb, 2:H:4])
                    nc.scalar.mul(ph[:, 4, r0:r1, 0:14], ph[:, 1, r0:r1, 0:14], 4.0)
                    nc.scalar.mul(ph[:, 5, r0:r1, 0:14], ph[:, 2, r0:r1, 0:14], 4.0)

            def emit_v_ops(icc, img, va, vb):
                """V-transform rows va..vb for one (icc, img).  DVE does the
                dependency-bearing work; GpSimd computes only leaves (E2s ->
                V3/V4) that feed the PE, never the DVE."""
                pg = img % 2
                ph, vt, sc_d = phs[icc][pg], vts[icc][pg], sc_ds[pg]
                q0 = ph[:, 0, va:vb, 0:TX]
                q1 = ph[:, 1, va:vb, 0:TX]
                q2 = ph[:, 2, va:vb, 0:TX]
                q3 = ph[:, 3, va:vb, 0:TX]
                q4 = ph[:, 0, va:vb, 1:PHB]
                q5 = ph[:, 1, va:vb, 1:PHB]
                b4 = ph[:, 4, va:vb, 0:TX]      # 4*E1[tx]
                d4 = ph[:, 5, va:vb, 0:TX]      # 4*E2[tx]
                Es = sc_d[:, 0, va:vb, :]
                F = sc_d[:, 1, va:vb, :]
                B = sc_d[:, 2, va:vb, :]
                D = sc_d[:, 3, va:vb, :]
                s = sc_d[:, 4, va:vb, :]
                t = sc_d[:, 5, va:vb, :]
                u = sc_d[:, 6, va:vb, :]
                v = lambda m: vt[:, m, va:vb, :]
                # emitted roughly in GEMM consumption order (m0, m1, ...)
                nc.vector.tensor_tensor(F, q4, q2, SUB)
                nc.vector.tensor_tensor(u, q0, q2, SUB)
                nc.vector.scalar_tensor_tensor(v(0), u, 4.0, F, MUL, ADD)  # 4q0-5q2+q4
                nc.vector.tensor_tensor(B, q3, q4, ADD)
                nc.vector.tensor_tensor(s, b4, d4, ADD)
                nc.vector.tensor_tensor(v(1), B, s, SUB)      # -4q1-4q2+q3+q4
                nc.vector.tensor_tensor(D, q3, q4, SUB)
                nc.vector.tensor_tensor(t, b4, d4, SUB)
                nc.vector.tensor_tensor(v(2), t, D, SUB)      # 4q1-4q2-q3+q4
                nc.vector.tensor_tensor(Es, q1, q3, SUB)
                nc.vector.scalar_tensor_tensor(v(3), Es, -2.0, F, MUL, ADD)
                nc.vector.scalar_tensor_tensor(v(4), Es, 2.0, F, MUL, ADD)
                nc.vector.scalar_tensor_tensor(u, Es, 4.0, q5, MUL, ADD)   # 4q1-4q3+q5
                nc.vector.tensor_tensor(v(5), u, q3, SUB)     # 4q1-5q3+q5

            def emit_v(img, halved=False):
                if halved:
                    for (va, vb, _, _) in halves:
                        for icc in range(2):
                            emit_v_ops(icc, img, va, vb)
                else:
                    for icc in range(2):
                        emit_v_ops(icc, img, 0, VR)

            def emit_out_transform(img, occ, c0, rows, dve_only):
                """Output transform for rows c0..c0+rows of one occ.  In the
                steady state GpSimd takes the leaves; for the tail (dve_only)
                everything runs on DVE per-chunk so nothing slow trails the
                last matmul."""
                pg = img % 2
                msb = msbs[(img * 2 + occ) % 2]
                ms = lambda m: msb[:, m, c0 : c0 + rows, :]
                s2 = sc2s[pg]
                sl = lambda i: s2[:, i, c0 : c0 + rows, :]
                I_, J_, G2, H2d, y3t, IJ = (sl(i) for i in range(6))
                H2 = sc2gh[:, 0, c0 : c0 + rows, :]
                yv = [sl(6), sl(7), sl(8), None]
                nc.vector.tensor_tensor(I_, ms(1), ms(2), ADD)
                nc.vector.tensor_tensor(J_, ms(3), ms(4), ADD)
                nc.vector.tensor_tensor(G2, ms(1), ms(2), SUB)
                nc.vector.tensor_tensor(H2, ms(3), ms(4), SUB)
                nc.vector.scalar_tensor_tensor(y3t, H2, 8.0, G2, MUL, ADD)
                nc.vector.scalar_tensor_tensor(yv[2], J_, 4.0, I_, MUL, ADD)
                if dve_only:
                    nc.vector.tensor_tensor(IJ, I_, J_, ADD)
                    nc.vector.tensor_tensor(yv[0], IJ, ms(0), ADD)
                    nc.vector.scalar_tensor_tensor(yv[1], H2, 2.0, G2, MUL, ADD)
                    yv3 = IJ
                    nc.vector.tensor_tensor(yv3, y3t, ms(5), ADD)
                else:
                    nc.vector.tensor_tensor(H2d, H2, H2, ADD)
                    nc.gpsimd.tensor_tensor(IJ, I_, J_, ADD)
                    nc.gpsimd.tensor_tensor(yv[0], IJ, ms(0), ADD)
                    nc.gpsimd.tensor_tensor(yv[1], G2, H2d, ADD)
                    yv3 = IJ  # reuse the IJ slab for the last phase
                    nc.gpsimd.tensor_tensor(yv3, y3t, ms(5), ADD)
                yv[3] = yv3
                for v_i in range(4):
                    nc.sync.dma_start(
                        out=y_d[img, occ * 128 : (occ + 1) * 128, v_i, c0 : c0 + rows, :],
                        in_=yv[v_i],
                    )

            def emit_compute_stage(img, act_inject=(), last=False):
                """GEMMs + evictions + output transform for one image.
                act_inject: list of (after_group_idx, fn) callbacks that emit
                extra ACT work between eviction groups so the ACT stream stays
                interleaved (splits for img+2 must not queue behind a full
                image of evictions)."""
                pg = img % 2
                inject = dict()
                for g, fn in act_inject:
                    inject.setdefault(g, []).append(fn)
                gidx = 0
                for occ in range(2):
                    msb = msbs[(img * 2 + occ) % 2]
                    for chunk in range(2):
                        c0 = chunk * 28
                        for m in range(6):
                            mt = pp.tile([128, 28, TX], f32, name="mt", tag="mt")
                            t = 0
                            for ky in range(3):
                                for icc in range(2):
                                    widx = ((occ * 6 + m) * 3 + ky) * 2 + icc
                                    nc.tensor.matmul(
                                        mt[:],
                                        w_sb[:, widx * 128 : (widx + 1) * 128],
                                        vts[icc][pg][:, m, c0 + ky : c0 + ky + 28, :],
                                        start=(t == 0),
                                        stop=(t == 5),
                                    )
                                    t += 1
                            nc.scalar.copy(out=msb[:, m, c0 : c0 + 28, :], in_=mt[:])
                        gidx += 1
                        for fn in inject.get(gidx, []):
                            fn()
                        if last:
                            emit_out_transform(img, occ, c0, 28, dve_only=True)
                    if not last:
                        emit_out_transform(img, occ, 0, H, dve_only=False)

            # ---- software-pipelined emission ----
            emit_x_dma(0)
            emit_w_dma(6, 30)    # rest of occ0 weights, behind img0's input
            emit_x_dma(1)
            emit_w_dma(36, 36)   # occ1 weights
            # interleave img0's splits by half so V (and the first GEMMs)
            # start as soon as half A of both iccs is ready
            for hv in halves:
                for icc in range(2):
                    emit_split_scale(0, icc, [hv])
            for icc in range(2):
                emit_split_scale(1, icc, None)
            emit_v(0, halved=True)
            emit_x_dma(2)
            emit_v(1)
            emit_compute_stage(
                0,
                act_inject=[
                    (2, lambda: emit_split_scale(2, 0, None)),
                    (3, lambda: emit_split_scale(2, 1, None)),
                ],
            )
            emit_x_dma(3)
            emit_v(2)
            emit_compute_stage(
                1,
                act_inject=[
                    (2, lambda: emit_split_scale(3, 0, None)),
                    (3, lambda: emit_split_scale(3, 1, None)),
                ],
            )
            emit_v(3)
            emit_compute_stage(2)
            emit_compute_stage(3, last=True)

    _split_waits(nc)
    return nc


def _prep_weight(weight: np.ndarray, mask: np.ndarray) -> np.ndarray:
    """[OC, IC, K, K] masked weight -> Winograd-transformed lhsT tiles
    [128ic, (occ,m,ky,icc)*128oc]."""
    G = np.array(
        [
            [1 / 4, 0, 0],
            [-1 / 6, -1 / 6, -1 / 6],
            [-1 / 6, 1 / 6, -1 / 6],
            [1 / 24, 1 / 12, 1 / 6],
            [1 / 24, -1 / 12, 1 / 6],
            [0, 0, 1],
        ],
        np.float32,
    )
    wm = (weight * mask).astype(np.float32)                  # [oc, ic, ky, kx]
    wp = np.einsum("mx,oikx->moik", G, wm)                   # [m, oc, ic, ky]
    t = wp.reshape(6, 2, 128, 2, 128, 3)                     # [m, occ, oc, icc, ic, ky]
    t = t.transpose(4, 1, 0, 5, 3, 2)                        # [ic, occ, m, ky, icc, oc]
    return np.ascontiguousarray(t.reshape(128, NW * 128).astype(np.float16))


def kernel(x: np.ndarray, weight: np.ndarray, mask: np.ndarray) -> np.ndarray:
    from concourse.bass_utils import run_bass_kernel_spmd

    x = np.asarray(x, dtype=np.float32)
    x16 = np.ascontiguousarray(x.astype(np.float16))
    w_host = _prep_weight(np.asarray(weight), np.asarray(mask))

    nc = _build_nc()
    in_maps = [
        {
            "x": np.ascontiguousarray(x16[c * IMG_PER_CORE : (c + 1) * IMG_PER_CORE]),
            "w": w_host,
        }
        for c in range(N_CORES)
    ]
    res = run_bass_kernel_spmd(nc, in_maps, core_ids=list(range(N_CORES)))
    out = np.empty_like(x)
    for c in range(N_CORES):
        yp = res.results[c]["y"]  # [4, C, 4, 56, 14] phase-planar fp16
        yi = np.transpose(yp, (0, 1, 3, 4, 2)).reshape(IMG_PER_CORE, C, H, H)
        out[c * IMG_PER_CORE : (c + 1) * IMG_PER_CORE] = yi.astype(np.float32)
    return out


# revision 29
# speedup vs baseline: 1.1204x; 1.0064x over previous
"""ExpanderConv2d as a Bass/Tile kernel for Trainium2, data-parallel over batch
across 8 NeuronCores.

Reference op: y = conv2d(x, weight * mask), N=32, C=256->256, 56x56, k=3,
stride 1, pad 1.

v10: 1D Winograd F(4,3) along W.  Each quad of 4 horizontal outputs costs 6
multiplies instead of 12, so the PE streams 2/3 the columns of the direct
9-tap formulation: 226k columns/core (~94us at 1 col/cycle) vs 451k (~188us).

Per img: x [128, 2icc, 56, 56] fp16 -> ACT phase-split into 4 column-phase
planes [58 rows, 15 blocks] + 3 ACT-prescaled planes (4*E0, 4*E1, 4*E2 —
exact in fp16) -> V[m] for the 6 Winograd components as 15 merged-icc DVE
ops (4D APs, one instruction covers both icc chunks; all but one are plain
tensor_tensor at the 2x rate) -> GEMMs M[m] = sum_{ky,icc} W'[m,ky,icc]^T
V[m] (PSUM fp32, moving dim = 28 rows x 14 tiles = 392) -> ACT evicts M to
SBUF fp16 -> output transform y[v] = At M on [56,14] slabs (DVE + GpSimd
leaves) -> fp16 phase-planar DMA; the host interleaves the 4 phases (pure
transpose) and upcasts.

Engine discipline learned from tracing: GpSimd ops are ~3x slower than DVE
and must never produce values DVE waits on; scratch buffers ping-pong per
image so no engine WAR-waits on another engine's reads; the ACT stream is
hand-interleaved so M evictions track the PE's PSUM-bank rotation; weights
stream just-in-time behind the first image's input.

Sharding: batch 32 -> 4 images per core; the transformed masked weight
(2.4 MB fp16, 72 [128x128] tiles) is replicated to every core.
"""

import numpy as np

N_CORES = 8
IMG_PER_CORE = 4
C = 256
H = 56
TX = 14          # winograd tiles per row (4 outputs each)
PHB = 15         # phase-plane blocks (E0/E1 need 15, E2/E3 use 14)
VR = 58          # V rows = padded rows
NW = 72          # weight tiles: occ(2) x m(6) x ky(3) x icc(2)


def _split_waits(nc, max_waits=1):
    """walrus in this container rejects instructions carrying more than one
    semaphore wait ("Too many sync wait commands").  Hoist the extra waits onto
    injected single-wait NoOps on the same engine just before the instruction —
    sem waits block the engine, so a chain of single waits is equivalent."""
    import concourse.mybir as mybir

    for f in nc.m.functions:
        for blk in f.blocks:
            out = []
            changed = False
            for inst in blk.instructions:
                si = inst.sync_info
                if si and si.on_wait and len(si.on_wait) > max_waits:
                    waits = list(si.on_wait)
                    extra, keep = waits[:-max_waits], waits[-max_waits:]
                    for j, w in enumerate(extra):
                        out.append(
                            mybir.InstNoOp(
                                name=f"{inst.name}-w{j}",
                                engine=inst.engine,
                                ins=[],
                                outs=[],
                                sync_info=mybir.SyncInfo(on_wait=[w], on_update=[]),
                                bass_nofuse=True,
                            )
                        )
                    si.on_wait = keep
                    changed = True
                out.append(inst)
            if changed:
                blk.instructions = out


def _build_nc():
    import concourse.bass as bass
    import concourse.mybir as mybir
    from concourse.tile import TileContext

    f32 = mybir.dt.float32
    f16 = mybir.dt.float16
    ADD = mybir.AluOpType.add
    SUB = mybir.AluOpType.subtract
    MUL = mybir.AluOpType.mult

    nc = bass.Bass("TRN2", target_bir_lowering=False, debug=False)
    x_d = nc.dram_tensor("x", [IMG_PER_CORE, C, H, H], f16, kind="ExternalInput").ap()
    w_d = nc.dram_tensor("w", [128, NW * 128], f16, kind="ExternalInput").ap()
    # y is stored phase-planar: y[img, c, v, h, tx] = out[img, c, h, 4*tx+v]
    y_d = nc.dram_tensor("y", [IMG_PER_CORE, C, 4, H, TX], f16, kind="ExternalOutput").ap()

    with TileContext(nc) as tc:
        with (
            tc.tile_pool(name="wpool", bufs=1) as wp,
            tc.tile_pool(name="xpool", bufs=1) as xp,
            tc.tile_pool(name="psum", bufs=8, space="PSUM") as pp,
            tc.tile_pool(name="msb", bufs=1) as mp,
            tc.tile_pool(name="scp", bufs=1) as scp,
        ):
            w_sb = wp.tile([128, NW * 128], f16, name="w_sb", tag="w_sb")

            def emit_w_dma(wq, wn):
                nc.scalar.dma_start(
                    out=w_sb[:, wq * 128 : (wq + wn) * 128],
                    in_=w_d[:, wq * 128 : (wq + wn) * 128],
                )

            # Only the first GEMM group's 6 tiles go out ahead of x; the rest
            # interleave with the x DMAs below so the weight bulk doesn't hog
            # the DMA queues while img0's input is in flight.
            emit_w_dma(0, 6)

            # Warm the PE clock gate (HAM) with throwaway matmuls on scratch
            # data while the first input/weight DMAs are in flight.
            warm = wp.tile([128, 392], f16, name="warm", tag="warm")
            nc.vector.memset(warm[:], 0.0)
            for i in range(10):
                warm_ps = pp.tile([128, 28, TX], f32, name="mt", tag="mt")
                nc.tensor.matmul(
                    warm_ps[:], warm[:, :128], warm[:, :392], start=True, stop=True
                )

            # Merged-icc buffers, ping-ponged per image.
            # ph planes: 0:E0 1:E1 2:E2 3:E3 4:b4=4*E1 5:d4=4*E2 6:a4=4*E0
            xrs = [xp.tile([128, 2, H, H], f16, name=f"xr{b}", tag=f"xr{b}") for b in range(2)]
            # icc is folded into the row dimension (rows 0..57 = icc0,
            # 58..115 = icc1) so merged-icc ops stay 3D (walrus rejects 4D).
            phs = [xp.tile([128, 7, 2 * VR, PHB], f16, name=f"ph{b}", tag=f"ph{b}") for b in range(2)]
            vts = [xp.tile([128, 6, 2 * VR, TX], f16, name=f"vt{b}", tag=f"vt{b}") for b in range(2)]
            # Zero the padding cells of the phase planes once; DMA/split/scale
            # only ever write interior rows/blocks, so they stay zero.
            for b in range(2):
                ph = phs[b]
                for pr in (0, VR - 1, VR, 2 * VR - 1):           # pad rows
                    nc.gpsimd.memset(ph[:, :, pr, :], 0.0)
                nc.gpsimd.memset(ph[:, 0, :, 0], 0.0)            # E0 left pad col
                nc.gpsimd.memset(ph[:, 6, :, 0], 0.0)            # a4 left pad col
                nc.gpsimd.memset(ph[:, 1, :, PHB - 1], 0.0)      # E1 right pad col

            # V scratch (DVE-private), 4 rotating slots per image parity:
            # slot0: F | slot1: r,D,D2,u | slot2: B,t,Es | slot3: s
            sc_ds = [scp.tile([128, 4, 2 * VR, TX], f16, name=f"sc_d{b}", tag=f"sc_d{b}") for b in range(2)]
            msbs = [mp.tile([128, 6, H, TX], f16, name=f"m{b}", tag=f"m{b}") for b in range(2)]
            # out-transform scratch: H2 is DVE-private; everything GpSimd
            # reads or DMA touches ping-pongs.
            sc2gh = scp.tile([128, 1, H, TX], f16, name="sc2gh", tag="sc2gh")
            sc2s = [scp.tile([128, 9, H, TX], f16, name=f"sc2{b}", tag=f"sc2{b}") for b in range(2)]

            # Row halves: half A covers V rows 0..29 (x rows 0..28),
            # half B V rows 30..57 (x rows 29..55).
            halves = [(0, 30, 0, 29), (30, VR, 29, H)]

            def emit_x_dma(img):
                xr = xrs[img % 2]
                for icc in range(2):
                    for (_, _, xa, xb) in halves:
                        nc.sync.dma_start(
                            out=xr[:, icc, xa:xb, :],
                            in_=x_d[img, icc * 128 : (icc + 1) * 128, xa:xb, :],
                        )

            def emit_split_scale(img, iccs, split_halves):
                """ACT: phase-split + prescaled planes.  iccs selects either
                one icc (startup staggering) or both merged."""
                pg = img % 2
                xr, ph = xrs[pg], phs[pg]
                hs = split_halves if split_halves is not None else [(0, VR, 0, H)]
                for (va, vb, xa, xb) in hs:
                    # phase p holds padded col c = 4*blk + p; data col w = c-1.
                    for icc in iccs:
                        r0 = (va + 1 if va == 0 else va) + icc * VR
                        r1 = (vb if vb != VR else VR - 1) + icc * VR
                        nc.scalar.copy(out=ph[:, 0, r0:r1, 1:PHB], in_=xr[:, icc, xa:xb, 3:H:4])
                        nc.scalar.copy(out=ph[:, 1, r0:r1, 0:14], in_=xr[:, icc, xa:xb, 0:H:4])
                        nc.scalar.copy(out=ph[:, 2, r0:r1, 0:14], in_=xr[:, icc, xa:xb, 1:H:4])
                        nc.scalar.copy(out=ph[:, 3, r0:r1, 0:14], in_=xr[:, icc, xa:xb, 2:H:4])
                    if iccs == [0, 1] and split_halves is None:
                        # pad rows are zero, so scaling the full plane is safe
                        nc.scalar.mul(ph[:, 4, :, 0:14], ph[:, 1, :, 0:14], 4.0)
                        nc.scalar.mul(ph[:, 5, :, 0:14], ph[:, 2, :, 0:14], 4.0)
                        nc.scalar.mul(ph[:, 6, :, 0:14], ph[:, 0, :, 0:14], 4.0)
                    else:
                        for icc in iccs:
                            r0, r1 = icc * VR + va, icc * VR + vb
                            nc.scalar.mul(ph[:, 4, r0:r1, 0:14], ph[:, 1, r0:r1, 0:14], 4.0)
                            nc.scalar.mul(ph[:, 5, r0:r1, 0:14], ph[:, 2, r0:r1, 0:14], 4.0)
                            nc.scalar.mul(ph[:, 6, r0:r1, 0:14], ph[:, 0, r0:r1, 0:14], 4.0)

            def emit_v(img, halved=False):
                """15 merged-icc DVE ops per row-range, in GEMM consumption
                order (m0 first)."""
                pg = img % 2
                ph, vt, sc_d = phs[pg], vts[pg], sc_ds[pg]
                if halved:
                    # startup: (half, icc) sub-blocks so the first GEMMs fire
                    # as soon as half A of both iccs exists
                    ranges = [(a + icc * VR, b + icc * VR)
                              for (a, b, _, _) in halves for icc in range(2)]

                else:
                    ranges = [(0, 2 * VR)]
                for (va, vb) in ranges:
                    q = lambda p: ph[:, p, va:vb, 0:TX]
                    q4 = ph[:, 0, va:vb, 1:PHB]
                    q5 = ph[:, 1, va:vb, 1:PHB]
                    b4 = ph[:, 4, va:vb, 0:TX]
                    d4 = ph[:, 5, va:vb, 0:TX]
                    a4 = ph[:, 6, va:vb, 0:TX]
                    sl = lambda k: sc_d[:, k, va:vb, :]
                    v = lambda m: vt[:, m, va:vb, :]
                    F, s1, s2, s3 = sl(0), sl(1), sl(2), sl(3)
                    nc.vector.tensor_tensor(F, q4, q(2), SUB)        # q4-q2
                    nc.vector.tensor_tensor(s1, a4, d4, SUB)         # r = 4q0-4q2
                    nc.vector.tensor_tensor(v(0), s1, F, ADD)        # 4q0-5q2+q4
                    nc.vector.tensor_tensor(s2, q(3), q4, ADD)       # B
                    nc.vector.tensor_tensor(s3, b4, d4, ADD)         # s
                    nc.vector.tensor_tensor(v(1), s2, s3, SUB)       # -4q1-4q2+q3+q4
                    nc.vector.tensor_tensor(s1, q(3), q4, SUB)       # D
                    nc.vector.tensor_tensor(s2, b4, d4, SUB)         # t
                    nc.vector.tensor_tensor(v(2), s2, s1, SUB)       # 4q1-4q2-q3+q4
                    nc.vector.tensor_tensor(s2, q(1), q(3), SUB)     # Es
                    nc.vector.tensor_tensor(s1, s2, s2, ADD)         # D2 = 2*Es
                    nc.vector.tensor_tensor(v(3), F, s1, SUB)        # -2q1-q2+2q3+q4
                    nc.vector.tensor_tensor(v(4), F, s1, ADD)        # 2q1-q2-2q3+q4
                    nc.vector.scalar_tensor_tensor(s1, s2, 4.0, q5, MUL, ADD)  # 4Es+q5
                    nc.vector.tensor_tensor(v(5), s1, q(3), SUB)     # 4q1-5q3+q5

            def emit_out_transform(img, occ, c0, rows, dve_only):
                """Output transform for rows c0..c0+rows of one occ.  In the
                steady state GpSimd takes the leaves; for the tail (dve_only)
                everything runs on DVE per-chunk so nothing slow trails the
                last matmul."""
                pg = img % 2
                msb = msbs[(img * 2 + occ) % 2]
                ms = lambda m: msb[:, m, c0 : c0 + rows, :]
                s2 = sc2s[pg]
                sl = lambda i: s2[:, i, c0 : c0 + rows, :]
                I_, J_, G2, H2d, y3t, IJ = (sl(i) for i in range(6))
                H2 = sc2gh[:, 0, c0 : c0 + rows, :]
                yv = [sl(6), sl(7), sl(8), None]
                nc.vector.tensor_tensor(I_, ms(1), ms(2), ADD)
                nc.vector.tensor_tensor(J_, ms(3), ms(4), ADD)
                nc.vector.tensor_tensor(G2, ms(1), ms(2), SUB)
                nc.vector.tensor_tensor(H2, ms(3), ms(4), SUB)
                nc.vector.scalar_tensor_tensor(y3t, H2, 8.0, G2, MUL, ADD)
                nc.vector.scalar_tensor_tensor(yv[2], J_, 4.0, I_, MUL, ADD)
                if dve_only:
                    nc.vector.tensor_tensor(IJ, I_, J_, ADD)
                    nc.vector.tensor_tensor(yv[0], IJ, ms(0), ADD)
                    nc.vector.scalar_tensor_tensor(yv[1], H2, 2.0, G2, MUL, ADD)
                    yv3 = IJ
                    nc.vector.tensor_tensor(yv3, y3t, ms(5), ADD)
                else:
                    nc.vector.tensor_tensor(H2d, H2, H2, ADD)
                    nc.gpsimd.tensor_tensor(IJ, I_, J_, ADD)
                    nc.gpsimd.tensor_tensor(yv[0], IJ, ms(0), ADD)
                    nc.gpsimd.tensor_tensor(yv[1], G2, H2d, ADD)
                    yv3 = IJ  # reuse the IJ slab for the last phase
                    nc.gpsimd.tensor_tensor(yv3, y3t, ms(5), ADD)
                yv[3] = yv3
                for v_i in range(4):
                    nc.sync.dma_start(
                        out=y_d[img, occ * 128 : (occ + 1) * 128, v_i, c0 : c0 + rows, :],
                        in_=yv[v_i],
                    )

            def emit_compute_stage(img, act_inject=(), last=False):
                """GEMMs + evictions + output transform for one image.
                act_inject: (after_group_idx, fn) callbacks emitting extra ACT
                work between eviction groups so next images' splits don't
                queue behind a full image of evictions."""
                pg = img % 2
                inject = dict()
                for g, fn in act_inject:
                    inject.setdefault(g, []).append(fn)
                gidx = 0
                for occ in range(2):
                    msb = msbs[(img * 2 + occ) % 2]
                    for chunk in range(2):
                        c0 = chunk * 28
                        for m in range(6):
                            mt = pp.tile([128, 28, TX], f32, name="mt", tag="mt")
                            t = 0
                            for ky in range(3):
                                for icc in range(2):
                                    widx = ((occ * 6 + m) * 3 + ky) * 2 + icc
                                    nc.tensor.matmul(
                                        mt[:],
                                        w_sb[:, widx * 128 : (widx + 1) * 128],
                                        vts[pg][:, m, icc * VR + c0 + ky : icc * VR + c0 + ky + 28, :],
                                        start=(t == 0),
                                        stop=(t == 5),
                                    )
                                    t += 1
                            nc.scalar.copy(out=msb[:, m, c0 : c0 + 28, :], in_=mt[:])
                        gidx += 1
                        for fn in inject.get(gidx, []):
                            fn()
                        if last and occ == 1:
                            emit_out_transform(img, occ, c0, 28, dve_only=True)
                    if not (last and occ == 1):
                        emit_out_transform(img, occ, 0, H, dve_only=False)

            # ---- software-pipelined emission ----
            emit_x_dma(0)
            emit_w_dma(6, 30)    # rest of occ0 weights, behind img0's input
            emit_x_dma(1)
            emit_w_dma(36, 36)   # occ1 weights
            # stagger img0's splits by (half, icc) so V starts early
            for hv in halves:
                for icc in range(2):
                    emit_split_scale(0, [icc], [hv])
            emit_split_scale(1, [0, 1], None)
            emit_v(0, halved=True)
            emit_x_dma(2)
            emit_v(1)
            emit_compute_stage(
                0,
                act_inject=[
                    (2, lambda: emit_split_scale(2, [0, 1], None)),
                ],
            )
            emit_x_dma(3)
            emit_v(2)
            emit_compute_stage(
                1,
                act_inject=[
                    (2, lambda: emit_split_scale(3, [0, 1], None)),
                ],
            )
            emit_v(3)
            emit_compute_stage(2)
            emit_compute_stage(3, last=True)

    _split_waits(nc)
    return nc


def _prep_weight(weight: np.ndarray, mask: np.ndarray) -> np.ndarray:
    """[OC, IC, K, K] masked weight -> Winograd-transformed lhsT tiles
    [128ic, (occ,m,ky,icc)*128oc]."""
    G = np.array(
        [
            [1 / 4, 0, 0],
            [-1 / 6, -1 / 6, -1 / 6],
            [-1 / 6, 1 / 6, -1 / 6],
            [1 / 24, 1 / 12, 1 / 6],
            [1 / 24, -1 / 12, 1 / 6],
            [0, 0, 1],
        ],
        np.float32,
    )
    wm = (weight * mask).astype(np.float32)                  # [oc, ic, ky, kx]
    wp = np.einsum("mx,oikx->moik", G, wm)                   # [m, oc, ic, ky]
    t = wp.reshape(6, 2, 128, 2, 128, 3)                     # [m, occ, oc, icc, ic, ky]
    t = t.transpose(4, 1, 0, 5, 3, 2)                        # [ic, occ, m, ky, icc, oc]
    return np.ascontiguousarray(t.reshape(128, NW * 128).astype(np.float16))


def kernel(x: np.ndarray, weight: np.ndarray, mask: np.ndarray) -> np.ndarray:
    from concourse.bass_utils import run_bass_kernel_spmd

    x = np.asarray(x, dtype=np.float32)
    x16 = np.ascontiguousarray(x.astype(np.float16))
    w_host = _prep_weight(np.asarray(weight), np.asarray(mask))

    nc = _build_nc()
    in_maps = [
        {
            "x": np.ascontiguousarray(x16[c * IMG_PER_CORE : (c + 1) * IMG_PER_CORE]),
            "w": w_host,
        }
        for c in range(N_CORES)
    ]
    res = run_bass_kernel_spmd(nc, in_maps, core_ids=list(range(N_CORES)))
    out = np.empty_like(x)
    for c in range(N_CORES):
        yp = res.results[c]["y"]  # [4, C, 4, 56, 14] phase-planar fp16
        yi = np.transpose(yp, (0, 1, 3, 4, 2)).reshape(IMG_PER_CORE, C, H, H)
        out[c * IMG_PER_CORE : (c + 1) * IMG_PER_CORE] = yi.astype(np.float32)
    return out


# revision 32
# speedup vs baseline: 1.1523x; 1.0285x over previous
"""ExpanderConv2d as a Bass/Tile kernel for Trainium2, data-parallel over batch
across 8 NeuronCores.

Reference op: y = conv2d(x, weight * mask), N=32, C=256->256, 56x56, k=3,
stride 1, pad 1.

v10: 1D Winograd F(4,3) along W.  Each quad of 4 horizontal outputs costs 6
multiplies instead of 12, so the PE streams 2/3 the columns of the direct
9-tap formulation: 226k columns/core (~94us at 1 col/cycle) vs 451k (~188us).

Per img: x [128, 2icc, 56, 56] fp16 -> ACT phase-split into 4 column-phase
planes [58 rows, 15 blocks] + 3 ACT-prescaled planes (4*E0, 4*E1, 4*E2 —
exact in fp16) -> V[m] for the 6 Winograd components as 15 merged-icc DVE
ops (4D APs, one instruction covers both icc chunks; all but one are plain
tensor_tensor at the 2x rate) -> GEMMs M[m] = sum_{ky,icc} W'[m,ky,icc]^T
V[m] (PSUM fp32, moving dim = 28 rows x 14 tiles = 392) -> ACT evicts M to
SBUF fp16 -> output transform y[v] = At M on [56,14] slabs (DVE + GpSimd
leaves) -> fp16 phase-planar DMA; the host interleaves the 4 phases (pure
transpose) and upcasts.

Engine discipline learned from tracing: GpSimd ops are ~3x slower than DVE
and must never produce values DVE waits on; scratch buffers ping-pong per
image so no engine WAR-waits on another engine's reads; the ACT stream is
hand-interleaved so M evictions track the PE's PSUM-bank rotation; weights
stream just-in-time behind the first image's input.

Sharding: batch 32 -> 4 images per core; the transformed masked weight
(2.4 MB fp16, 72 [128x128] tiles) is replicated to every core.
"""

import numpy as np

N_CORES = 8
IMG_PER_CORE = 4
C = 256
H = 56
TX = 14          # winograd tiles per row (4 outputs each)
PHB = 15         # phase-plane blocks (E0/E1 need 15, E2/E3 use 14)
VR = 58          # V rows = padded rows
NW = 72          # weight tiles: occ(2) x m(6) x ky(3) x icc(2)


def _split_waits(nc, max_waits=1):
    """walrus in this container rejects instructions carrying more than one
    semaphore wait ("Too many sync wait commands").  Hoist the extra waits onto
    injected single-wait NoOps on the same engine just before the instruction —
    sem waits block the engine, so a chain of single waits is equivalent."""
    import concourse.mybir as mybir

    for f in nc.m.functions:
        for blk in f.blocks:
            out = []
            changed = False
            for inst in blk.instructions:
                si = inst.sync_info
                if si and si.on_wait and len(si.on_wait) > max_waits:
                    waits = list(si.on_wait)
                    extra, keep = waits[:-max_waits], waits[-max_waits:]
                    for j, w in enumerate(extra):
                        out.append(
                            mybir.InstNoOp(
                                name=f"{inst.name}-w{j}",
                                engine=inst.engine,
                                ins=[],
                                outs=[],
                                sync_info=mybir.SyncInfo(on_wait=[w], on_update=[]),
                                bass_nofuse=True,
                            )
                        )
                    si.on_wait = keep
                    changed = True
                out.append(inst)
            if changed:
                blk.instructions = out


def _build_nc():
    import concourse.bass as bass
    import concourse.mybir as mybir
    from concourse.tile import TileContext

    f32 = mybir.dt.float32
    f16 = mybir.dt.float16
    ADD = mybir.AluOpType.add
    SUB = mybir.AluOpType.subtract
    MUL = mybir.AluOpType.mult

    nc = bass.Bass("TRN2", target_bir_lowering=False, debug=False)
    x_d = nc.dram_tensor("x", [IMG_PER_CORE, C, H, H], f16, kind="ExternalInput").ap()
    w_d = nc.dram_tensor("w", [128, NW * 128], f16, kind="ExternalInput").ap()
    # y is stored phase-planar: y[img, c, v, h, tx] = out[img, c, h, 4*tx+v]
    y_d = nc.dram_tensor("y", [IMG_PER_CORE, C, 4, H, TX], f16, kind="ExternalOutput").ap()

    with TileContext(nc) as tc:
        with (
            tc.tile_pool(name="wpool", bufs=1) as wp,
            tc.tile_pool(name="xpool", bufs=1) as xp,
            tc.tile_pool(name="psum", bufs=8, space="PSUM") as pp,
            tc.tile_pool(name="msb", bufs=1) as mp,
            tc.tile_pool(name="scp", bufs=1) as scp,
        ):
            w_sb = wp.tile([128, NW * 128], f16, name="w_sb", tag="w_sb")

            def emit_w_dma(wq, wn):
                nc.scalar.dma_start(
                    out=w_sb[:, wq * 128 : (wq + wn) * 128],
                    in_=w_d[:, wq * 128 : (wq + wn) * 128],
                )

            # Only the first GEMM group's 6 tiles go out ahead of x; the rest
            # interleave with the x DMAs below so the weight bulk doesn't hog
            # the DMA queues while img0's input is in flight.
            emit_w_dma(0, 6)

            # Warm the PE clock gate (HAM) with throwaway matmuls on scratch
            # data while the first input/weight DMAs are in flight.
            warm = wp.tile([128, 392], f16, name="warm", tag="warm")
            nc.vector.memset(warm[:], 0.0)
            for i in range(10):
                warm_ps = pp.tile([128, 28, TX], f32, name="mt", tag="mt")
                nc.tensor.matmul(
                    warm_ps[:], warm[:, :128], warm[:, :392], start=True, stop=True
                )

            # Merged-icc buffers, ping-ponged per image.
            # ph planes: 0:E0 1:E1 2:E2 3:E3 4:b4=4*E1 5:d4=4*E2 6:a4=4*E0
            xrs = [xp.tile([128, 2, H, H], f16, name=f"xr{b}", tag=f"xr{b}") for b in range(2)]
            # icc is folded into the row dimension (rows 0..57 = icc0,
            # 58..115 = icc1) so merged-icc ops stay 3D (walrus rejects 4D).
            phs = [xp.tile([128, 7, 2 * VR, PHB], f16, name=f"ph{b}", tag=f"ph{b}") for b in range(2)]
            vts = [xp.tile([128, 6, 2 * VR, TX], f16, name=f"vt{b}", tag=f"vt{b}") for b in range(2)]
            # Zero the padding cells of the phase planes once; DMA/split/scale
            # only ever write interior rows/blocks, so they stay zero.
            for b in range(2):
                ph = phs[b]
                for pr in (0, VR - 1, VR, 2 * VR - 1):           # pad rows
                    nc.gpsimd.memset(ph[:, :, pr, :], 0.0)
                nc.gpsimd.memset(ph[:, 0, :, 0], 0.0)            # E0 left pad col
                nc.gpsimd.memset(ph[:, 6, :, 0], 0.0)            # a4 left pad col
                nc.gpsimd.memset(ph[:, 1, :, PHB - 1], 0.0)      # E1 right pad col

            # V scratch (DVE-private), 4 rotating slots per image parity:
            # slot0: F | slot1: r,D,D2,u | slot2: B,t,Es | slot3: s
            sc_ds = [scp.tile([128, 4, 2 * VR, TX], f16, name=f"sc_d{b}", tag=f"sc_d{b}") for b in range(2)]
            msbs = [mp.tile([128, 6, H, TX], f16, name=f"m{b}", tag=f"m{b}") for b in range(2)]
            # out-transform scratch: H2 is DVE-private; everything GpSimd
            # reads or DMA touches ping-pongs.
            sc2gh = scp.tile([128, 1, H, TX], f16, name="sc2gh", tag="sc2gh")
            sc2s = [scp.tile([128, 9, H, TX], f16, name=f"sc2{b}", tag=f"sc2{b}") for b in range(2)]

            # Row halves: half A covers V rows 0..29 (x rows 0..28),
            # half B V rows 30..57 (x rows 29..55).
            halves = [(0, 30, 0, 29), (30, VR, 29, H)]

            def emit_x_dma(img):
                xr = xrs[img % 2]
                for icc in range(2):
                    for (_, _, xa, xb) in halves:
                        nc.sync.dma_start(
                            out=xr[:, icc, xa:xb, :],
                            in_=x_d[img, icc * 128 : (icc + 1) * 128, xa:xb, :],
                        )

            def emit_split_scale(img, iccs, split_halves):
                """ACT: phase-split + prescaled planes.  iccs selects either
                one icc (startup staggering) or both merged."""
                pg = img % 2
                xr, ph = xrs[pg], phs[pg]
                hs = split_halves if split_halves is not None else [(0, VR, 0, H)]
                for (va, vb, xa, xb) in hs:
                    # phase p holds padded col c = 4*blk + p; data col w = c-1.
                    for icc in iccs:
                        r0 = (va + 1 if va == 0 else va) + icc * VR
                        r1 = (vb if vb != VR else VR - 1) + icc * VR
                        nc.scalar.copy(out=ph[:, 0, r0:r1, 1:PHB], in_=xr[:, icc, xa:xb, 3:H:4])
                        nc.scalar.copy(out=ph[:, 1, r0:r1, 0:14], in_=xr[:, icc, xa:xb, 0:H:4])
                        nc.scalar.copy(out=ph[:, 2, r0:r1, 0:14], in_=xr[:, icc, xa:xb, 1:H:4])
                        nc.scalar.copy(out=ph[:, 3, r0:r1, 0:14], in_=xr[:, icc, xa:xb, 2:H:4])
                    if iccs == [0, 1] and split_halves is None:
                        # pad rows are zero, so scaling the full plane is safe
                        nc.scalar.mul(ph[:, 4, :, 0:14], ph[:, 1, :, 0:14], 4.0)
                        nc.scalar.mul(ph[:, 5, :, 0:14], ph[:, 2, :, 0:14], 4.0)
                        nc.scalar.mul(ph[:, 6, :, 0:14], ph[:, 0, :, 0:14], 4.0)
                    else:
                        for icc in iccs:
                            r0, r1 = icc * VR + va, icc * VR + vb
                            nc.scalar.mul(ph[:, 4, r0:r1, 0:14], ph[:, 1, r0:r1, 0:14], 4.0)
                            nc.scalar.mul(ph[:, 5, r0:r1, 0:14], ph[:, 2, r0:r1, 0:14], 4.0)
                            nc.scalar.mul(ph[:, 6, r0:r1, 0:14], ph[:, 0, r0:r1, 0:14], 4.0)

            def emit_v(img, halved=False):
                """15 merged-icc DVE ops per row-range, in GEMM consumption
                order (m0 first)."""
                pg = img % 2
                ph, vt, sc_d = phs[pg], vts[pg], sc_ds[pg]
                if halved:
                    # startup: (half, icc) sub-blocks so the first GEMMs fire
                    # as soon as half A of both iccs exists
                    ranges = [(a + icc * VR, b + icc * VR)
                              for (a, b, _, _) in halves for icc in range(2)]

                else:
                    ranges = [(0, 2 * VR)]
                for (va, vb) in ranges:
                    q = lambda p: ph[:, p, va:vb, 0:TX]
                    q4 = ph[:, 0, va:vb, 1:PHB]
                    q5 = ph[:, 1, va:vb, 1:PHB]
                    b4 = ph[:, 4, va:vb, 0:TX]
                    d4 = ph[:, 5, va:vb, 0:TX]
                    a4 = ph[:, 6, va:vb, 0:TX]
                    sl = lambda k: sc_d[:, k, va:vb, :]
                    v = lambda m: vt[:, m, va:vb, :]
                    F, s1, s2, s3 = sl(0), sl(1), sl(2), sl(3)
                    # emitted in GEMM consumption order: m1, m2, m3, m4, m0, m5
                    nc.vector.tensor_tensor(s2, q(3), q4, ADD)       # B
                    nc.vector.tensor_tensor(s3, b4, d4, ADD)         # s
                    nc.vector.tensor_tensor(v(1), s2, s3, SUB)       # -4q1-4q2+q3+q4
                    nc.vector.tensor_tensor(s1, q(3), q4, SUB)       # D
                    nc.vector.tensor_tensor(s2, b4, d4, SUB)         # t
                    nc.vector.tensor_tensor(v(2), s2, s1, SUB)       # 4q1-4q2-q3+q4
                    nc.vector.tensor_tensor(F, q4, q(2), SUB)        # q4-q2
                    nc.vector.tensor_tensor(s3, q(1), q(3), SUB)     # Es
                    nc.vector.tensor_tensor(s1, s3, s3, ADD)         # D2 = 2*Es
                    nc.vector.tensor_tensor(v(3), F, s1, SUB)        # -2q1-q2+2q3+q4
                    nc.vector.tensor_tensor(v(4), F, s1, ADD)        # 2q1-q2-2q3+q4
                    nc.vector.tensor_tensor(s2, a4, d4, SUB)         # r = 4q0-4q2
                    nc.vector.tensor_tensor(v(0), s2, F, ADD)        # 4q0-5q2+q4
                    nc.vector.scalar_tensor_tensor(s1, s3, 4.0, q5, MUL, ADD)  # 4Es+q5
                    nc.vector.tensor_tensor(v(5), s1, q(3), SUB)     # 4q1-5q3+q5

            def emit_out_transform(img, occ, c0, rows, dve_only):
                """Output transform for rows c0..c0+rows of one occ.  In the
                steady state GpSimd takes the leaves; for the tail (dve_only)
                everything runs on DVE per-chunk with the opposite parity's
                scratch so nothing WAR-waits on GpSimd after the last
                matmul."""
                pg = img % 2
                msb = msbs[(img * 2 + occ) % 2]
                ms = lambda m: msb[:, m, c0 : c0 + rows, :]
                s2 = sc2s[1 - pg] if dve_only else sc2s[pg]
                sl = lambda i: s2[:, i, c0 : c0 + rows, :]
                I_, J_, G2, H2d, y3t, IJ = (sl(i) for i in range(6))
                H2 = sc2gh[:, 0, c0 : c0 + rows, :]
                yv = [sl(6), sl(7), sl(8), None]
                nc.vector.tensor_tensor(I_, ms(1), ms(2), ADD)
                nc.vector.tensor_tensor(J_, ms(3), ms(4), ADD)
                nc.vector.tensor_tensor(G2, ms(1), ms(2), SUB)
                nc.vector.tensor_tensor(H2, ms(3), ms(4), SUB)
                nc.vector.scalar_tensor_tensor(y3t, H2, 8.0, G2, MUL, ADD)
                nc.vector.scalar_tensor_tensor(yv[2], J_, 4.0, I_, MUL, ADD)
                if dve_only:
                    nc.vector.tensor_tensor(IJ, I_, J_, ADD)
                    nc.vector.tensor_tensor(yv[0], IJ, ms(0), ADD)
                    nc.vector.scalar_tensor_tensor(yv[1], H2, 2.0, G2, MUL, ADD)
                    yv3 = IJ
                    nc.vector.tensor_tensor(yv3, y3t, ms(5), ADD)
                else:
                    nc.vector.tensor_tensor(H2d, H2, H2, ADD)
                    nc.gpsimd.tensor_tensor(IJ, I_, J_, ADD)
                    nc.gpsimd.tensor_tensor(yv[0], IJ, ms(0), ADD)
                    nc.gpsimd.tensor_tensor(yv[1], G2, H2d, ADD)
                    yv3 = IJ  # reuse the IJ slab for the last phase
                    nc.gpsimd.tensor_tensor(yv3, y3t, ms(5), ADD)
                yv[3] = yv3
                for v_i in range(4):
                    nc.sync.dma_start(
                        out=y_d[img, occ * 128 : (occ + 1) * 128, v_i, c0 : c0 + rows, :],
                        in_=yv[v_i],
                    )

            def compute_steps(img, last=False):
                """Generator: 4 steps, one per (occ, chunk) — each emits the
                6 GEMM groups + evictions; the occ's output transform is
                emitted with its second chunk.  Stepping from the top level
                interleaves next images' splits into the ACT stream at the
                right points while keeping program-order dataflow correct."""
                pg = img % 2
                for occ in range(2):
                    msb = msbs[(img * 2 + occ) % 2]
                    for chunk in range(2):
                        c0 = chunk * 28
                        # m0/m5 last: their msb slabs are read by GpSimd
                        # (yv0/yv3), so their evictions in the NEXT image
                        # tolerate GpSimd lag without stalling ACT.
                        for m in (1, 2, 3, 4, 0, 5):
                            mt = pp.tile([128, 28, TX], f32, name="mt", tag="mt")
                            t = 0
                            for ky in range(3):
                                for icc in range(2):
                                    widx = ((occ * 6 + m) * 3 + ky) * 2 + icc
                                    nc.tensor.matmul(
                                        mt[:],
                                        w_sb[:, widx * 128 : (widx + 1) * 128],
                                        vts[pg][:, m, icc * VR + c0 + ky : icc * VR + c0 + ky + 28, :],
                                        start=(t == 0),
                                        stop=(t == 5),
                                    )
                                    t += 1
                            nc.scalar.copy(out=msb[:, m, c0 : c0 + 28, :], in_=mt[:])
                        if last and occ == 1:
                            emit_out_transform(img, occ, c0, 28, dve_only=True)
                        elif chunk == 1:
                            emit_out_transform(img, occ, 0, H, dve_only=False)
                        yield

            # ---- software-pipelined emission ----
            def run(gen, n):
                for _ in range(n):
                    next(gen, None)

            emit_x_dma(0)
            emit_w_dma(6, 30)    # rest of occ0 weights, behind img0's input
            emit_x_dma(1)
            emit_w_dma(36, 36)   # occ1 weights
            # stagger img0's splits by (half, icc) so V starts early
            for hv in halves:
                for icc in range(2):
                    emit_split_scale(0, [icc], [hv])
            emit_v(0, halved=True)
            emit_x_dma(2)
            g0 = compute_steps(0)
            run(g0, 2)                       # img0 occ0
            # ss/V for img k may only be emitted once compute(k-2) has fully
            # emitted its vt reads (emission order defines dataflow).
            emit_split_scale(1, [0, 1], None)
            emit_v(1)
            run(g0, 2)                       # img0 occ1
            emit_x_dma(3)
            emit_split_scale(2, [0, 1], None)
            emit_v(2)
            run(compute_steps(1), 4)
            emit_split_scale(3, [0, 1], None)
            emit_v(3)
            run(compute_steps(2), 4)
            run(compute_steps(3, last=True), 4)

    _split_waits(nc)
    return nc


def _prep_weight(weight: np.ndarray, mask: np.ndarray) -> np.ndarray:
    """[OC, IC, K, K] masked weight -> Winograd-transformed lhsT tiles
    [128ic, (occ,m,ky,icc)*128oc]."""
    G = np.array(
        [
            [1 / 4, 0, 0],
            [-1 / 6, -1 / 6, -1 / 6],
            [-1 / 6, 1 / 6, -1 / 6],
            [1 / 24, 1 / 12, 1 / 6],
            [1 / 24, -1 / 12, 1 / 6],
            [0, 0, 1],
        ],
        np.float32,
    )
    wm = (weight * mask).astype(np.float32)                  # [oc, ic, ky, kx]
    wp = np.einsum("mx,oikx->moik", G, wm)                   # [m, oc, ic, ky]
    t = wp.reshape(6, 2, 128, 2, 128, 3)                     # [m, occ, oc, icc, ic, ky]
    t = t.transpose(4, 1, 0, 5, 3, 2)                        # [ic, occ, m, ky, icc, oc]
    return np.ascontiguousarray(t.reshape(128, NW * 128).astype(np.float16))


def kernel(x: np.ndarray, weight: np.ndarray, mask: np.ndarray) -> np.ndarray:
    from concourse.bass_utils import run_bass_kernel_spmd

    x = np.asarray(x, dtype=np.float32)
    x16 = np.ascontiguousarray(x.astype(np.float16))
    w_host = _prep_weight(np.asarray(weight), np.asarray(mask))

    nc = _build_nc()
    in_maps = [
        {
            "x": np.ascontiguousarray(x16[c * IMG_PER_CORE : (c + 1) * IMG_PER_CORE]),
            "w": w_host,
        }
        for c in range(N_CORES)
    ]
    res = run_bass_kernel_spmd(nc, in_maps, core_ids=list(range(N_CORES)))
    out = np.empty_like(x)
    for c in range(N_CORES):
        yp = res.results[c]["y"]  # [4, C, 4, 56, 14] phase-planar fp16
        yi = np.transpose(yp, (0, 1, 3, 4, 2)).reshape(IMG_PER_CORE, C, H, H)
        out[c * IMG_PER_CORE : (c + 1) * IMG_PER_CORE] = yi.astype(np.float32)
    return out
